# revision 1
# baseline (speedup 1.0000x reference)
"""GAT (2x GATConv + global_mean_pool + MLP) on 8 Trainium2 NeuronCores.

Strategy (sharding_hint: 1D node partition, replicated weights):
  - dst nodes partitioned 8 ways (1250/core, padded to 10 tiles of 128 slots);
    edges sorted by dst, grouped into per-(core,tile) blocks of 128.
  - Layer 1 aggregates x[src] (128 wide) instead of h[src] (1024 wide):
    sum_e ex*(x W1) == (sum_e ex*x) W1 per head -> 8x less gather traffic.
    Attention logits via fused vectors V = W @ a (alpha = x @ V).
  - Segment softmax-sum via one-hot indicator matmuls on the tensor engine
    (PSUM accumulation); normalization after projection (commutes per head).
  - Only exchange: AllGather of per-core [1280, 528] fp16 packed table
    (h2p = elu(out1) @ W2, plus alpha2_src bitcast) + [1280, 8] f32 alpha2_dst,
    and a tiny AllReduce of pooled per-graph sums. MLP replicated.
All float math in f32 on device (fp16 only for the exchanged table).
"""
import os
import sys
import numpy as np

for _p in ("/opt/trn_rl_repo",):
    if os.path.isdir(_p) and _p not in sys.path:
        sys.path.insert(0, _p)

N = 10000
B = 16
NCORES = 8
P = 128
NPC = 1250                  # nodes per core
TPC = 10                    # dst tiles per core
NPAD = 10112                # 79 * 128
NTILES_A = 79
NEG = 0.2
L2ROWS = NCORES * TPC * P   # 10240

_PROGRAM_CACHE = {}
LAST_PROFILE = {}
DEBUG_DUMPS = False


def _preprocess(edge_index, batch):
    src = np.concatenate([np.asarray(edge_index[0]), np.arange(N)]).astype(np.int64)
    dst = np.concatenate([np.asarray(edge_index[1]), np.arange(N)]).astype(np.int64)
    order = np.argsort(dst, kind='stable')
    src, dst = src[order], dst[order]

    core_of = dst // NPC
    local = dst - core_of * NPC
    tile_of = local // P
    seg_of = (local - tile_of * P).astype(np.float32)

    counts = np.zeros((NCORES, TPC), dtype=np.int64)
    np.add.at(counts, (core_of, tile_of), 1)
    bt = int(np.ceil(counts.max() / P))
    bt = max(bt, 1)

    src_m = np.zeros((NCORES, TPC, P, bt), dtype=np.int32)
    dst_m = np.zeros((NCORES, TPC, P, bt), dtype=np.int32)
    seg_m = np.full((NCORES, TPC, P, bt), -1.0, dtype=np.float32)

    flat_group = core_of * TPC + tile_of
    grp_start = np.searchsorted(flat_group, np.arange(NCORES * TPC), 'left')
    rank = np.arange(len(flat_group)) - grp_start[flat_group]
    blk = rank // P
    part = rank % P
    co = core_of.astype(np.int64)
    ti = tile_of.astype(np.int64)
    src_m[co, ti, part, blk] = src.astype(np.int32)
    dst_m[co, ti, part, blk] = dst.astype(np.int32)
    seg_m[co, ti, part, blk] = seg_of

    # L2 table rows: node n lives at core*1280 + (n - core*1250)
    node = np.arange(N, dtype=np.int64)
    cn = node // NPC
    l2row = (cn * TPC * P + (node - cn * NPC)).astype(np.int32)
    srcl2_m = l2row[src_m.reshape(-1)].reshape(src_m.shape)
    dstl2_m = l2row[dst_m.reshape(-1)].reshape(dst_m.shape)

    batch = np.asarray(batch).astype(np.int64)
    gid = np.full((NCORES, TPC, P), -1.0, dtype=np.float32)
    for c in range(NCORES):
        lo = c * NPC
        hi = min(lo + NPC, N)
        vals = batch[lo:hi].astype(np.float32)
        g = gid[c].reshape(-1)
        g[:hi - lo] = vals

    cnt = np.zeros(B, np.float32)
    np.add.at(cnt, batch, 1.0)
    recip_cnt16 = (1.0 / np.maximum(cnt, 1.0)).astype(np.float32).reshape(16, 1)

    return dict(bt=bt, src_m=src_m, dst_m=dst_m, seg_m=seg_m,
                srcl2_m=srcl2_m, dstl2_m=dstl2_m, gid=gid,
                recip_cnt16=recip_cnt16)


def _build_program(bt, upto='full', repeat=1):
    import concourse.bacc as bacc
    import concourse.bass as bass
    import concourse.mybir as mybir
    import concourse.tile as tile
    from concourse.masks import make_identity

    f32 = mybir.dt.float32
    f16 = mybir.dt.float16
    i32 = mybir.dt.int32
    AF = mybir.ActivationFunctionType
    OP = mybir.AluOpType
    IOA = bass.IndirectOffsetOnAxis

    _ORDER = ['none', 'A', 'B', 'AG', 'E', 'AR', 'F', 'full']
    def _inc(s):
        return _ORDER.index(upto if upto != 'full' else 'F') >= _ORDER.index(s)

    nc = bacc.Bacc("TRN2", target_bir_lowering=False, debug=False,
                   enable_asserts=False, num_devices=NCORES)

    # ---------------- inputs ----------------
    t_x = nc.dram_tensor("x_pad", [NPAD, P], f32, kind="ExternalInput")
    t_W1 = nc.dram_tensor("W1", [P, 1024], f32, kind="ExternalInput")
    t_V1 = nc.dram_tensor("V1", [P, 16], f32, kind="ExternalInput")
    t_W2 = nc.dram_tensor("W2", [1024, 512], f32, kind="ExternalInput")
    t_V2 = nc.dram_tensor("V2", [1024, 16], f32, kind="ExternalInput")
    t_b1r = nc.dram_tensor("b1_rep", [P, 1024], f32, kind="ExternalInput")
    t_b2r = nc.dram_tensor("b2_rep", [P, 512], f32, kind="ExternalInput")
    t_iota128 = nc.dram_tensor("iota128", [P, P], f32, kind="ExternalInput")
    t_iota16 = nc.dram_tensor("iota16", [P, 16], f32, kind="ExternalInput")
    t_rc16 = nc.dram_tensor("recip_cnt16", [16, 1], f32, kind="ExternalInput")
    t_fc1w = nc.dram_tensor("fc1_w", [512, 32], f32, kind="ExternalInput")
    t_fc1b = nc.dram_tensor("fc1_b", [32, 1], f32, kind="ExternalInput")
    t_fc2w = nc.dram_tensor("fc2_w", [32, 10], f32, kind="ExternalInput")
    t_fc2br = nc.dram_tensor("fc2_b_rep", [16, 10], f32, kind="ExternalInput")
    t_srcm = nc.dram_tensor("src_m", [TPC, P, bt], i32, kind="ExternalInput")
    t_dstm = nc.dram_tensor("dst_m", [TPC, P, bt], i32, kind="ExternalInput")
    t_segm = nc.dram_tensor("seg_m", [TPC, P, bt], f32, kind="ExternalInput")
    t_srcl2 = nc.dram_tensor("srcl2_m", [TPC, P, bt], i32, kind="ExternalInput")
    t_dstl2 = nc.dram_tensor("dstl2_m", [TPC, P, bt], i32, kind="ExternalInput")
    t_gid = nc.dram_tensor("gid_m", [TPC, P], f32, kind="ExternalInput")
    t_chain = nc.dram_tensor("chain", [16, 10], f32, kind="ExternalInput")

    t_out = nc.dram_tensor("out", [16, 10], f32, kind="ExternalOutput")
    dbg = {}
    if DEBUG_DUMPS:
        dbg['xa'] = nc.dram_tensor("dbg_xa", [NPAD, 136], f32, kind="ExternalOutput")
        dbg['a1d'] = nc.dram_tensor("dbg_a1d", [NPAD, 8], f32, kind="ExternalOutput")
        dbg['h2p'] = nc.dram_tensor("dbg_h2p", [TPC * P, 512], f16, kind="ExternalOutput")
        dbg['a2'] = nc.dram_tensor("dbg_a2", [TPC * P, 16], f32, kind="ExternalOutput")
        dbg['pool'] = nc.dram_tensor("dbg_pool", [16, 512], f32, kind="ExternalOutput")
        dbg['poolr'] = nc.dram_tensor("dbg_poolr", [16, 512], f32, kind="ExternalOutput")
        dbg['h2pf'] = nc.dram_tensor("dbg_h2pf", [L2ROWS, 512], f16, kind="ExternalOutput")
        dbg['a2f'] = nc.dram_tensor("dbg_a2f", [L2ROWS, 16], f32, kind="ExternalOutput")

    with tile.TileContext(nc) as tc:
        with (
            tc.tile_pool(name="const", bufs=1) as csb,
            tc.tile_pool(name="dram", bufs=1, space="DRAM") as dr,
        ):
            # DRAM staging
            xa_tab = dr.tile([NPAD, 136], f32)          # [x | alpha1_src]
            a1d_tab = dr.tile([NPAD, 8], f32)           # alpha1_dst
            h2p_loc = dr.tile([TPC * P, 512], f16)
            a2_loc = dr.tile([TPC * P, 16], f32)        # [a2src | a2dst]
            a2s_tab = dr.tile([L2ROWS, 8], f32)
            a2d_tab = dr.tile([L2ROWS, 8], f32)
            pool_in = dr.tile([16, 512], f32)

            # resident constants
            ident = csb.tile([P, P], f32)
            make_identity(nc, ident[:])
            iota128 = csb.tile([P, P], f32)
            nc.sync.dma_start(out=iota128[:], in_=t_iota128[:])
            iota16 = csb.tile([P, 16], f32)
            nc.sync.dma_start(out=iota16[:], in_=t_iota16[:])
            W1sb = csb.tile([P, 1024], f32)
            nc.sync.dma_start(out=W1sb[:], in_=t_W1[:])
            V1sb = csb.tile([P, 16], f32)
            nc.sync.dma_start(out=V1sb[:], in_=t_V1[:])
            W2sb = []
            V2sb = []
            for c in range(8):
                w2c = csb.tile([P, 512], f32, name=f"w2c{c}")
                nc.sync.dma_start(out=w2c[:], in_=t_W2[c * P:(c + 1) * P, :])
                W2sb.append(w2c)
                v2c = csb.tile([P, 16], f32, name=f"v2c{c}")
                nc.sync.dma_start(out=v2c[:], in_=t_V2[c * P:(c + 1) * P, :])
                V2sb.append(v2c)
            b1r = csb.tile([P, 1024], f32)
            nc.sync.dma_start(out=b1r[:], in_=t_b1r[:])
            b2r = csb.tile([P, 512], f32)
            nc.sync.dma_start(out=b2r[:], in_=t_b2r[:])
            rc16 = csb.tile([16, 1], f32)
            nc.sync.dma_start(out=rc16[:], in_=t_rc16[:])
            chain_sb = csb.tile([16, 10], f32)
            nc.sync.dma_start(out=chain_sb[:], in_=t_chain[:])

            # bulk copy x into xa_tab[:, 0:128]
            nc.sync.dma_start(out=xa_tab[:, 0:P], in_=t_x[:])

            for _rep in range(repeat):
                h2p_full = dr.tile([L2ROWS, 512], f16, addr_space="Shared",
                                   name=f"h2p_full{_rep}")
                a2_full = dr.tile([L2ROWS, 16], f32, addr_space="Shared",
                                  name=f"a2_full{_rep}")
                pool_out = dr.tile([16, 512], f32, addr_space="Shared",
                                   name=f"pool_out{_rep}")
                # ---------------- Phase A: alpha1 tables (replicated) ---------
                with (
                    tc.tile_pool(name="pa_sb", bufs=3) as asb,
                    tc.tile_pool(name="pa_ps", bufs=2, space="PSUM") as aps,
                ):
                    for t in range(NTILES_A if _inc('A') else 0):
                        sl = slice(t * P, (t + 1) * P)
                        xt = asb.tile([P, P], f32, name="xt")
                        nc.sync.dma_start(out=xt[:], in_=t_x[sl, :])
                        xT_ps = aps.tile([P, P], f32, name="xT_ps")
                        nc.tensor.transpose(out=xT_ps[:], in_=xt[:], identity=ident[:])
                        xT = asb.tile([P, P], f32, name="xT")
                        nc.vector.tensor_copy(out=xT[:], in_=xT_ps[:])
                        al_ps = aps.tile([P, 16], f32, name="al_ps")
                        nc.tensor.matmul(al_ps[:], lhsT=xT[:], rhs=V1sb[:],
                                         start=True, stop=True)
                        al = asb.tile([P, 16], f32, name="al")
                        nc.vector.tensor_copy(out=al[:], in_=al_ps[:])
                        nc.sync.dma_start(out=xa_tab[sl, P:P + 8], in_=al[:, 0:8])
                        nc.sync.dma_start(out=a1d_tab[sl, :], in_=al[:, 8:16])

                # ---------------- Phase B: L1 aggregation + finalize ----------
                with (
                    tc.tile_pool(name="pb_sb", bufs=4) as bsb,
                    tc.tile_pool(name="pb_big", bufs=3) as bbig,
                    tc.tile_pool(name="pb_fin", bufs=2) as bfin,
                    tc.tile_pool(name="pb_ps", bufs=1, space="PSUM") as bps,
                ):
                    for t in range(TPC):
                        srcm = bsb.tile([P, bt], i32, name="srcm")
                        nc.sync.dma_start(out=srcm[:], in_=t_srcm[t])
                        dstm = bsb.tile([P, bt], i32, name="dstm")
                        nc.sync.dma_start(out=dstm[:], in_=t_dstm[t])
                        segm = bsb.tile([P, bt], f32, name="segm")
                        nc.sync.dma_start(out=segm[:], in_=t_segm[t])

                        axden = bps.tile([P, 1536], f32, name="axden", bufs=1)
                        for b in range(bt):
                            xag = bbig.tile([P, 136], f32, name="xag")
                            nc.gpsimd.indirect_dma_start(
                                out=xag[:], out_offset=None, in_=xa_tab[:],
                                in_offset=IOA(ap=srcm[:, b:b + 1], axis=0))
                            adg = bsb.tile([P, 8], f32, name="adg")
                            nc.gpsimd.indirect_dma_start(
                                out=adg[:], out_offset=None, in_=a1d_tab[:],
                                in_offset=IOA(ap=dstm[:, b:b + 1], axis=0))
                            oh = bbig.tile([P, P], f32, name="oh")
                            nc.vector.tensor_scalar(
                                out=oh[:], in0=iota128[:], scalar1=segm[:, b:b + 1],
                                scalar2=None, op0=OP.is_equal)
                            e = bsb.tile([P, 8], f32, name="e")
                            nc.vector.tensor_add(out=e[:], in0=xag[:, P:P + 8],
                                                 in1=adg[:])
                            es = bsb.tile([P, 8], f32, name="es")
                            nc.vector.tensor_scalar_mul(out=es[:], in0=e[:],
                                                        scalar1=NEG)
                            lr = bsb.tile([P, 8], f32, name="lr")
                            nc.vector.tensor_max(out=lr[:], in0=e[:], in1=es[:])
                            ex = bsb.tile([P, 8], f32, name="ex")
                            nc.scalar.activation(ex[:], lr[:], AF.Exp)
                            msg = bbig.tile([P, 1024], f32, name="msg")
                            msg_v = msg[:].rearrange("p (h c) -> p h c", h=8)
                            xg_b = xag[:, 0:P].unsqueeze(1).broadcast_to([P, 8, P])
                            ex_b = ex[:].unsqueeze(2).broadcast_to([P, 8, P])
                            nc.vector.tensor_tensor(out=msg_v, in0=xg_b, in1=ex_b,
                                                    op=OP.mult)
                            st = (b == 0)
                            sp = (b == bt - 1)
                            nc.tensor.matmul(axden[:, 0:512], lhsT=oh[:],
                                             rhs=msg[:, 0:512], start=st, stop=sp)
                            nc.tensor.matmul(axden[:, 512:1024], lhsT=oh[:],
                                             rhs=msg[:, 512:1024], start=st, stop=sp)
                            nc.tensor.matmul(axden[:, 1024:1032], lhsT=oh[:],
                                             rhs=ex[:], start=st, stop=sp)

                        # ---- finalize tile t
                        den = bsb.tile([P, 8], f32, name="den")
                        nc.vector.tensor_scalar_max(out=den[:],
                                                    in0=axden[:, 1024:1032],
                                                    scalar1=1e-30)
                        rec = bsb.tile([P, 8], f32, name="rec")
                        nc.vector.reciprocal(out=rec[:], in_=den[:])
                        axsb = bfin.tile([P, 1024], f32, name="axsb")
                        nc.vector.tensor_copy(out=axsb[:], in_=axden[:, 0:1024])

                        y = bfin.tile([P, 1024], f32, name="y")
                        for h in range(8):
                            hs = slice(h * P, (h + 1) * P)
                            tps = bps.tile([P, P], f32, name="tps", tag="pp", bufs=3)
                            nc.tensor.transpose(out=tps[:], in_=axsb[:, hs],
                                                identity=ident[:])
                            tsb = bfin.tile([P, P], f32, name="tsb", tag="tsb", bufs=3)
                            nc.vector.tensor_copy(out=tsb[:], in_=tps[:])
                            o1p = bps.tile([P, P], f32, name="o1p", tag="pp", bufs=3)
                            nc.tensor.matmul(o1p[:], lhsT=tsb[:], rhs=W1sb[:, hs],
                                             start=True, stop=True)
                            nc.vector.tensor_scalar_mul(out=y[:, hs], in0=o1p[:],
                                                        scalar1=rec[:, h:h + 1])
                        y2 = bfin.tile([P, 1024], f32, name="y2")
                        nc.vector.tensor_add(out=y2[:], in0=y[:], in1=b1r[:])
                        # elu = (max(y2,0) - 1) + exp(min(y2,0))
                        neg = bfin.tile([P, 1024], f32, name="neg")
                        nc.vector.tensor_scalar_min(out=neg[:], in0=y2[:], scalar1=0.0)
                        en = bfin.tile([P, 1024], f32, name="en")
                        nc.scalar.activation(en[:], neg[:], AF.Exp)
                        pm1 = bfin.tile([P, 1024], f32, name="pm1")
                        nc.vector.tensor_scalar(out=pm1[:], in0=y2[:], scalar1=0.0,
                                                scalar2=-1.0, op0=OP.max, op1=OP.add)
                        e1 = bfin.tile([P, 1024], f32, name="e1")
                        nc.vector.tensor_add(out=e1[:], in0=pm1[:], in1=en[:])

                        h2p_ps = bps.tile([P, 512], f32, name="h2p_ps", bufs=1)
                        a2_ps = bps.tile([P, 16], f32, name="a2_ps", bufs=1)
                        for c in range(8):
                            cs = slice(c * P, (c + 1) * P)
                            tps2 = bps.tile([P, P], f32, name="tps2", tag="pp", bufs=3)
                            nc.tensor.transpose(out=tps2[:], in_=e1[:, cs],
                                                identity=ident[:])
                            tsb2 = bfin.tile([P, P], f32, name="tsb2", tag="tsb", bufs=3)
                            nc.vector.tensor_copy(out=tsb2[:], in_=tps2[:])
                            nc.tensor.matmul(h2p_ps[:], lhsT=tsb2[:], rhs=W2sb[c][:],
                                             start=(c == 0), stop=(c == 7))
                            nc.tensor.matmul(a2_ps[:], lhsT=tsb2[:], rhs=V2sb[c][:],
                                             start=(c == 0), stop=(c == 7))
                        pk = bfin.tile([P, 512], f16, name="pk")
                        nc.vector.tensor_copy(out=pk[:], in_=h2p_ps[:])
                        a2sb = bsb.tile([P, 16], f32, name="a2sb")
                        nc.vector.tensor_copy(out=a2sb[:], in_=a2_ps[:])
                        sl = slice(t * P, (t + 1) * P)
                        nc.sync.dma_start(out=h2p_loc[sl, :], in_=pk[:])
                        nc.sync.dma_start(out=a2_loc[sl, :], in_=a2sb[:])

                # ---------------- AllGather exchange --------------------------
                if _inc('AG'):
                    nc.gpsimd.collective_compute(
                        "AllGather", mybir.AluOpType.bypass,
                        replica_groups=[list(range(NCORES))],
                        ins=[h2p_loc[:].opt()], outs=[h2p_full[:].opt()])
                    nc.gpsimd.collective_compute(
                        "AllGather", mybir.AluOpType.bypass,
                        replica_groups=[list(range(NCORES))],
                        ins=[a2_loc[:].opt()], outs=[a2_full[:].opt()])
                    # split [a2src | a2dst] into separate gather tables
                    nc.sync.dma_start(out=a2s_tab[:], in_=a2_full[:, 0:8])
                    nc.sync.dma_start(out=a2d_tab[:], in_=a2_full[:, 8:16])

                # ---------------- Phase E: L2 aggregation + pool --------------
                with (
                    tc.tile_pool(name="pe_sb", bufs=4) as esb,
                    tc.tile_pool(name="pe_big", bufs=3) as ebig,
                    tc.tile_pool(name="pe_fin", bufs=2) as efin,
                    tc.tile_pool(name="pe_ps", bufs=1, space="PSUM") as eps,
                ):
                    pool_ps = eps.tile([16, 512], f32, name="pool_ps", bufs=1)
                    for t in range(TPC if _inc('E') else 0):
                        srcm = esb.tile([P, bt], i32, name="srcm2")
                        nc.sync.dma_start(out=srcm[:], in_=t_srcl2[t])
                        dstm = esb.tile([P, bt], i32, name="dstm2")
                        nc.sync.dma_start(out=dstm[:], in_=t_dstl2[t])
                        segm = esb.tile([P, bt], f32, name="segm2")
                        nc.sync.dma_start(out=segm[:], in_=t_segm[t])

                        ahden = eps.tile([P, 520], f32, name="ahden", bufs=2)
                        for b in range(bt):
                            hg = ebig.tile([P, 512], f16, name="hg")
                            nc.gpsimd.indirect_dma_start(
                                out=hg[:], out_offset=None, in_=h2p_full[:],
                                in_offset=IOA(ap=srcm[:, b:b + 1], axis=0))
                            asg = esb.tile([P, 8], f32, name="asg2")
                            nc.gpsimd.indirect_dma_start(
                                out=asg[:], out_offset=None, in_=a2s_tab[:],
                                in_offset=IOA(ap=srcm[:, b:b + 1], axis=0))
                            adg = esb.tile([P, 8], f32, name="adg2")
                            nc.gpsimd.indirect_dma_start(
                                out=adg[:], out_offset=None, in_=a2d_tab[:],
                                in_offset=IOA(ap=dstm[:, b:b + 1], axis=0))
                            oh = ebig.tile([P, P], f32, name="oh2")
                            nc.vector.tensor_scalar(
                                out=oh[:], in0=iota128[:], scalar1=segm[:, b:b + 1],
                                scalar2=None, op0=OP.is_equal)
                            e = esb.tile([P, 8], f32, name="e2")
                            nc.vector.tensor_add(out=e[:], in0=asg[:], in1=adg[:])
                            es = esb.tile([P, 8], f32, name="es2")
                            nc.vector.tensor_scalar_mul(out=es[:], in0=e[:],
                                                        scalar1=NEG)
                            lr = esb.tile([P, 8], f32, name="lr2")
                            nc.vector.tensor_max(out=lr[:], in0=e[:], in1=es[:])
                            ex = esb.tile([P, 8], f32, name="ex2")
                            nc.scalar.activation(ex[:], lr[:], AF.Exp)
                            msg = ebig.tile([P, 512], f32, name="msg2")
                            msg_v = msg[:].rearrange("p (h c) -> p h c", h=8)
                            hg_v = hg[:, 0:512].rearrange("p (h c) -> p h c", h=8)
                            ex_b = ex[:].unsqueeze(2).broadcast_to([P, 8, 64])
                            nc.vector.tensor_tensor(out=msg_v, in0=hg_v, in1=ex_b,
                                                    op=OP.mult)
                            st = (b == 0)
                            sp = (b == bt - 1)
                            nc.tensor.matmul(ahden[:, 0:512], lhsT=oh[:],
                                             rhs=msg[:], start=st, stop=sp)
                            nc.tensor.matmul(ahden[:, 512:520], lhsT=oh[:],
                                             rhs=ex[:], start=st, stop=sp)

                        # ---- finalize tile t
                        den = esb.tile([P, 8], f32, name="den2")
                        nc.vector.tensor_scalar_max(out=den[:],
                                                    in0=ahden[:, 512:520],
                                                    scalar1=1e-30)
                        rec = esb.tile([P, 8], f32, name="rec2")
                        nc.vector.reciprocal(out=rec[:], in_=den[:])
                        y = efin.tile([P, 512], f32, name="yl2")
                        for h in range(8):
                            hs = slice(h * 64, (h + 1) * 64)
                            nc.vector.tensor_scalar_mul(out=y[:, hs],
                                                        in0=ahden[:, hs],
                                                        scalar1=rec[:, h:h + 1])
                        y2 = efin.tile([P, 512], f32, name="y2l2")
                        nc.vector.tensor_add(out=y2[:], in0=y[:], in1=b2r[:])
                        neg = efin.tile([P, 512], f32, name="negl2")
                        nc.vector.tensor_scalar_min(out=neg[:], in0=y2[:], scalar1=0.0)
                        en = efin.tile([P, 512], f32, name="enl2")
                        nc.scalar.activation(en[:], neg[:], AF.Exp)
                        pm1 = efin.tile([P, 512], f32, name="pm1l2")
                        nc.vector.tensor_scalar(out=pm1[:], in0=y2[:], scalar1=0.0,
                                                scalar2=-1.0, op0=OP.max, op1=OP.add)
                        e2t = efin.tile([P, 512], f32, name="e2t")
                        nc.vector.tensor_add(out=e2t[:], in0=pm1[:], in1=en[:])

                        gidt = esb.tile([P, 1], f32, name="gidt")
                        nc.sync.dma_start(out=gidt[:], in_=t_gid[t, :, None])
                        gone = esb.tile([P, 16], f32, name="gone")
                        nc.vector.tensor_scalar(out=gone[:], in0=iota16[:],
                                                scalar1=gidt[:, 0:1], scalar2=None,
                                                op0=OP.is_equal)
                        nc.tensor.matmul(pool_ps[:], lhsT=gone[:], rhs=e2t[:],
                                         start=(t == 0), stop=(t == TPC - 1))

                    # pool -> dram bounce
                    if _inc('E'):
                        pool_sb = esb.tile([16, 512], f32, name="pool_sb")
                        nc.vector.tensor_copy(out=pool_sb[:], in_=pool_ps[:])
                        nc.sync.dma_start(out=pool_in[:], in_=pool_sb[:])

                if _inc('AR'):
                    nc.gpsimd.collective_compute(
                        "AllReduce", mybir.AluOpType.add,
                        replica_groups=[list(range(NCORES))],
                        ins=[pool_in[:].opt()], outs=[pool_out[:].opt()])

                if DEBUG_DUMPS:
                    nc.sync.dma_start(out=dbg['xa'][:], in_=xa_tab[:])
                    nc.sync.dma_start(out=dbg['a1d'][:], in_=a1d_tab[:])
                    nc.sync.dma_start(out=dbg['h2p'][:], in_=h2p_loc[:])
                    nc.sync.dma_start(out=dbg['a2'][:], in_=a2_loc[:])
                    nc.sync.dma_start(out=dbg['pool'][:], in_=pool_in[:])
                    nc.sync.dma_start(out=dbg['poolr'][:], in_=pool_out[:])
                    nc.sync.dma_start(out=dbg['h2pf'][:], in_=h2p_full[:])
                    nc.sync.dma_start(out=dbg['a2f'][:], in_=a2_full[:])

            # ---------------- Phase F: MLP (replicated) -------------------
            if not _inc('F'):
                nc.sync.dma_start(out=t_out[:], in_=chain_sb[:])
            if _inc('F'):
             with (
                tc.tile_pool(name="pf_sb", bufs=1) as fsb,
                tc.tile_pool(name="pf_ps", bufs=1, space="PSUM") as fps,
             ):
                psb = fsb.tile([16, 512], f32, name="psb")
                nc.sync.dma_start(out=psb[:], in_=pool_out[:])
                gt = fsb.tile([16, 512], f32, name="gt")
                nc.vector.tensor_scalar_mul(out=gt[:], in0=psb[:],
                                            scalar1=rc16[:, 0:1])
                fc1c = []
                for c in range(4):
                    fw = fsb.tile([P, 32], f32, name=f"fc1c{c}")
                    nc.sync.dma_start(out=fw[:], in_=t_fc1w[c * P:(c + 1) * P, :])
                    fc1c.append(fw)
                fb1 = fsb.tile([32, 1], f32, name="fb1")
                nc.sync.dma_start(out=fb1[:], in_=t_fc1b[:])
                fw2 = fsb.tile([32, 10], f32, name="fw2")
                nc.sync.dma_start(out=fw2[:], in_=t_fc2w[:])
                fb2 = fsb.tile([16, 10], f32, name="fb2")
                nc.sync.dma_start(out=fb2[:], in_=t_fc2br[:])

                fc1_ps = fps.tile([32, 16], f32, name="fc1_ps")
                for c in range(4):
                    gtt_ps = fps.tile([P, 16], f32, name="gtt_ps", tag="gtt")
                    nc.tensor.transpose(out=gtt_ps[:], in_=gt[:, c * P:(c + 1) * P],
                                        identity=ident[0:16, 0:16])
                    gtt = fsb.tile([P, 16], f32, name="gtt_sb", tag="gtts")
                    nc.vector.tensor_copy(out=gtt[:], in_=gtt_ps[:])
                    nc.tensor.matmul(fc1_ps[:], lhsT=fc1c[c][:],
                                     rhs=gtt[:],
                                     start=(c == 0), stop=(c == 3))
                y1 = fsb.tile([32, 16], f32, name="y1")
                nc.vector.tensor_scalar_add(out=y1[:], in0=fc1_ps[:],
                                            scalar1=fb1[:, 0:1])
                neg1 = fsb.tile([32, 16], f32, name="neg1")
                nc.vector.tensor_scalar_min(out=neg1[:], in0=y1[:], scalar1=0.0)
                en1 = fsb.tile([32, 16], f32, name="en1")
                nc.scalar.activation(en1[:], neg1[:], AF.Exp)
                pm11 = fsb.tile([32, 16], f32, name="pm11")
                nc.vector.tensor_scalar(out=pm11[:], in0=y1[:], scalar1=0.0,
                                        scalar2=-1.0, op0=OP.max, op1=OP.add)
                g2 = fsb.tile([32, 16], f32, name="g2")
                nc.vector.tensor_add(out=g2[:], in0=pm11[:], in1=en1[:])

                fc2_ps = fps.tile([16, 10], f32, name="fc2_ps")
                nc.tensor.matmul(fc2_ps[:], lhsT=g2[:], rhs=fw2[:],
                                 start=True, stop=True)
                osb = fsb.tile([16, 10], f32, name="osb")
                nc.vector.tensor_add(out=osb[:], in0=fc2_ps[:], in1=fb2[:])
                nc.sync.dma_start(out=t_out[:], in_=osb[:])

    nc.compile()
    return nc


def kernel(x, edge_index, batch, W1, att_src1, att_dst1, b1,
           W2, att_src2, att_dst2, b2, fc1_w, fc1_b, fc2_w, fc2_b,
           _trace=False):
    from concourse.bass_utils import run_bass_kernel_spmd
    if _trace:
        try:
            import profile_util
            profile_util.install()
        except Exception:
            pass

    x = np.asarray(x, np.float32)
    W1 = np.asarray(W1, np.float32)
    W2 = np.asarray(W2, np.float32)
    a_s1 = np.asarray(att_src1, np.float32)
    a_d1 = np.asarray(att_dst1, np.float32)
    a_s2 = np.asarray(att_src2, np.float32)
    a_d2 = np.asarray(att_dst2, np.float32)

    pp = _preprocess(np.asarray(edge_index), np.asarray(batch))
    bt = pp['bt']

    if bt not in _PROGRAM_CACHE:
        _PROGRAM_CACHE[bt] = _build_program(bt)
    nc = _PROGRAM_CACHE[bt]

    x_pad = np.zeros((NPAD, P), np.float32)
    x_pad[:N] = x
    V1 = np.zeros((P, 16), np.float32)
    V2 = np.zeros((1024, 16), np.float32)
    for h in range(8):
        V1[:, h] = W1[:, h * P:(h + 1) * P] @ a_s1[h]
        V1[:, 8 + h] = W1[:, h * P:(h + 1) * P] @ a_d1[h]
        V2[:, h] = W2[:, h * 64:(h + 1) * 64] @ a_s2[h]
        V2[:, 8 + h] = W2[:, h * 64:(h + 1) * 64] @ a_d2[h]

    common = {
        "x_pad": x_pad,
        "W1": W1,
        "V1": V1,
        "W2": W2,
        "V2": V2,
        "b1_rep": np.tile(np.asarray(b1, np.float32)[None, :], (P, 1)),
        "b2_rep": np.tile(np.asarray(b2, np.float32)[None, :], (P, 1)),
        "iota128": np.tile(np.arange(P, dtype=np.float32)[None, :], (P, 1)),
        "iota16": np.tile(np.arange(16, dtype=np.float32)[None, :], (P, 1)),
        "recip_cnt16": pp['recip_cnt16'],
        "fc1_w": np.asarray(fc1_w, np.float32),
        "fc1_b": np.asarray(fc1_b, np.float32).reshape(32, 1),
        "fc2_w": np.asarray(fc2_w, np.float32),
        "fc2_b_rep": np.tile(np.asarray(fc2_b, np.float32)[None, :], (16, 1)),
    }
    in_maps = []
    for c in range(NCORES):
        m = dict(common)
        m["src_m"] = pp['src_m'][c]
        m["dst_m"] = pp['dst_m'][c]
        m["seg_m"] = pp['seg_m'][c]
        m["srcl2_m"] = pp['srcl2_m'][c]
        m["dstl2_m"] = pp['dstl2_m'][c]
        m["gid_m"] = pp['gid'][c]
        m["chain"] = np.zeros((16, 10), np.float32)
        in_maps.append(m)

    res = run_bass_kernel_spmd(nc, in_maps, list(range(NCORES)),
                               trace=bool(_trace))
    LAST_PROFILE.clear()
    LAST_PROFILE['exec_time_ns'] = res.exec_time_ns
    LAST_PROFILE['results'] = res
    return np.asarray(res.results[0]["out"], np.float32)



# revision 5
# speedup vs baseline: 1.3524x; 1.3524x over previous
"""GAT (2x GATConv + global_mean_pool + MLP) on 8 Trainium2 NeuronCores.

Strategy (sharding_hint: 1D node partition, replicated weights):
  - dst nodes partitioned 8 ways (1250/core, 10 tiles of 128 slots);
    edges sorted by dst, packed into per-(core,tile) blocks of 128.
  - Layer 1 aggregates x[src] (128 wide) instead of h[src] (1024 wide):
    sum_e ex*(x W1) == (sum_e ex*x) W1 per head -> 8x less gather traffic.
    Attention logits via fused vectors V = W @ a (alpha = x @ V).
  - All aggregation math in fp16 (PE 4x faster than f32; gathers half the
    bytes); PSUM accumulation stays f32.
  - alpha_dst is NOT gathered per edge: dst rows of a tile are contiguous,
    so load the tile's [128,8] alphas once and permute per block with a
    one-hot matmul (ohT built from a host-side transposed segment table
    via a rank-1 broadcast matmul + is_equal).
  - Only exchange: AllGather of per-core [1280, 528] fp16 packed table
    (h2 = elu(out1) @ W2 | alpha2_src), plus a tiny AllReduce of pooled
    per-graph sums. alpha2_dst stays core-local. MLP replicated.
"""
import os
import sys
import numpy as np

for _p in ("/opt/trn_rl_repo",):
    if os.path.isdir(_p) and _p not in sys.path:
        sys.path.insert(0, _p)

N = 10000
B = 16
NCORES = 8
P = 128
NPC = 1250                  # nodes per core
TPC = 10                    # dst tiles per core
NPAD = 10112                # 79 * 128
NTILES_A = 79
NEG = 0.2
L2ROWS = NCORES * TPC * P   # 10240
XAW = 136                   # xa table row: 128 x + 8 alpha_src (f16)
H2W = 528                   # h2a table row: 512 h2 + 8 alpha2_src + 8 pad

_PROGRAM_CACHE = {}
LAST_PROFILE = {}
DEBUG_DUMPS = False


def _preprocess(edge_index, batch):
    src = np.concatenate([np.asarray(edge_index[0]), np.arange(N)]).astype(np.int64)
    dst = np.concatenate([np.asarray(edge_index[1]), np.arange(N)]).astype(np.int64)
    order = np.argsort(dst, kind='stable')
    src, dst = src[order], dst[order]

    core_of = dst // NPC
    local = dst - core_of * NPC
    tile_of = local // P
    seg_of = (local - tile_of * P).astype(np.float32)

    counts = np.zeros((NCORES, TPC), dtype=np.int64)
    np.add.at(counts, (core_of, tile_of), 1)
    bt = int(np.ceil(counts.max() / P))
    bt = max(bt, 1)

    src_m = np.zeros((NCORES, TPC, P, bt), dtype=np.int32)
    seg_m = np.full((NCORES, TPC, P, bt), -1.0, dtype=np.float32)

    flat_group = core_of * TPC + tile_of
    grp_start = np.searchsorted(flat_group, np.arange(NCORES * TPC), 'left')
    rank = np.arange(len(flat_group)) - grp_start[flat_group]
    blk = rank // P
    part = rank % P
    co = core_of.astype(np.int64)
    ti = tile_of.astype(np.int64)
    src_m[co, ti, part, blk] = src.astype(np.int32)
    seg_m[co, ti, part, blk] = seg_of

    # transposed segment table: segT[c, t, b*128 + j] = seg_m[c, t, j, b]
    segT_m = np.ascontiguousarray(
        seg_m.transpose(0, 1, 3, 2).reshape(NCORES, TPC, bt * P)).astype(np.float16)

    # L2 table rows: node n lives at core*1280 + (n - core*1250)
    node = np.arange(N, dtype=np.int64)
    cn = node // NPC
    l2row = (cn * TPC * P + (node - cn * NPC)).astype(np.int32)
    srcl2_m = l2row[src_m.reshape(-1)].reshape(src_m.shape)

    # per-tile dst node ids (rows of a1d_tab): core c, tile t, slot p
    dsttile = np.zeros((NCORES, TPC, P, 1), dtype=np.int32)
    for c in range(NCORES):
        for t in range(TPC):
            ids = c * NPC + t * P + np.arange(P)
            ids = np.minimum(ids, (c + 1) * NPC - 1)   # clamp pad slots
            dsttile[c, t, :, 0] = ids

    batch = np.asarray(batch).astype(np.int64)
    gid = np.full((NCORES, TPC, P), -1.0, dtype=np.float32)
    for c in range(NCORES):
        lo = c * NPC
        hi = min(lo + NPC, N)
        vals = batch[lo:hi].astype(np.float32)
        g = gid[c].reshape(-1)
        g[:hi - lo] = vals

    cnt = np.zeros(B, np.float32)
    np.add.at(cnt, batch, 1.0)
    recip_cnt16 = (1.0 / np.maximum(cnt, 1.0)).astype(np.float32).reshape(16, 1)

    return dict(bt=bt, src_m=src_m, seg_m=seg_m, segT_m=segT_m,
                srcl2_m=srcl2_m, dsttile=dsttile, gid=gid,
                recip_cnt16=recip_cnt16)


def _build_program(bt, upto='full', repeat=1):
    import concourse.bacc as bacc
    import concourse.bass as bass
    import concourse.mybir as mybir
    import concourse.tile as tile
    from concourse.masks import make_identity

    f32 = mybir.dt.float32
    f16 = mybir.dt.float16
    i32 = mybir.dt.int32
    AF = mybir.ActivationFunctionType
    OP = mybir.AluOpType
    IOA = bass.IndirectOffsetOnAxis

    _ORDER = ['none', 'A', 'B', 'AG', 'E', 'AR', 'F', 'full']
    def _inc(s):
        return _ORDER.index(upto if upto != 'full' else 'F') >= _ORDER.index(s)

    nc = bacc.Bacc("TRN2", target_bir_lowering=False, debug=False,
                   enable_asserts=False, num_devices=NCORES)

    # ---------------- inputs ----------------
    t_x16 = nc.dram_tensor("x16", [NPAD, P], f16, kind="ExternalInput")
    t_xT16 = nc.dram_tensor("xT16", [P, NPAD], f16, kind="ExternalInput")
    t_W1 = nc.dram_tensor("W1_16", [P, 1024], f16, kind="ExternalInput")
    t_V1 = nc.dram_tensor("V1_16", [P, 16], f16, kind="ExternalInput")
    t_W2 = nc.dram_tensor("W2_16", [1024, 512], f16, kind="ExternalInput")
    t_V2 = nc.dram_tensor("V2_16", [1024, 16], f16, kind="ExternalInput")
    t_b1c = nc.dram_tensor("b1cols", [P, 8], f32, kind="ExternalInput")
    t_b2r = nc.dram_tensor("b2rep16", [P, 512], f16, kind="ExternalInput")
    t_iota128 = nc.dram_tensor("iota128_16", [P, P], f16, kind="ExternalInput")
    t_iota16 = nc.dram_tensor("iota16_16", [P, 16], f16, kind="ExternalInput")
    t_iotacol = nc.dram_tensor("iotacol", [P, 1], f32, kind="ExternalInput")
    t_rc16 = nc.dram_tensor("recip_cnt16", [16, 1], f32, kind="ExternalInput")
    t_fc1w = nc.dram_tensor("fc1_w", [512, 32], f32, kind="ExternalInput")
    t_fc1b = nc.dram_tensor("fc1_b", [32, 1], f32, kind="ExternalInput")
    t_fc2w = nc.dram_tensor("fc2_w", [32, 10], f32, kind="ExternalInput")
    t_fc2br = nc.dram_tensor("fc2_b_rep", [16, 10], f32, kind="ExternalInput")
    t_srcm = nc.dram_tensor("src_m", [TPC, P, bt], i32, kind="ExternalInput")
    t_segm = nc.dram_tensor("seg_m", [TPC, P, bt], f32, kind="ExternalInput")
    t_segT = nc.dram_tensor("segT_m", [TPC, bt * P], f16, kind="ExternalInput")
    t_srcl2 = nc.dram_tensor("srcl2_m", [TPC, P, bt], i32, kind="ExternalInput")
    t_dstt = nc.dram_tensor("dsttile", [TPC, P, 1], i32, kind="ExternalInput")
    t_gid = nc.dram_tensor("gid_m", [TPC, P], f32, kind="ExternalInput")
    t_chain = nc.dram_tensor("chain", [16, 10], f32, kind="ExternalInput")

    t_out = nc.dram_tensor("out", [16, 10], f32, kind="ExternalOutput")
    dbg = {}
    if DEBUG_DUMPS:
        dbg['xa'] = nc.dram_tensor("dbg_xa", [NPAD, XAW], f16, kind="ExternalOutput")
        dbg['a1d'] = nc.dram_tensor("dbg_a1d", [NPAD, 8], f16, kind="ExternalOutput")
        dbg['h2a'] = nc.dram_tensor("dbg_h2a", [TPC * P, H2W], f16, kind="ExternalOutput")
        dbg['a2d'] = nc.dram_tensor("dbg_a2d", [TPC * P, 8], f16, kind="ExternalOutput")
        dbg['pool'] = nc.dram_tensor("dbg_pool", [16, 512], f32, kind="ExternalOutput")

    with tile.TileContext(nc) as tc:
        with (
            tc.tile_pool(name="const", bufs=1) as csb,
            tc.tile_pool(name="dram", bufs=1, space="DRAM") as dr,
        ):
            # DRAM staging
            xa_tab = dr.tile([NPAD, XAW], f16)          # [x | alpha1_src]
            a1d_tab = dr.tile([NPAD, 8], f16)           # alpha1_dst
            h2a_loc = dr.tile([TPC * P, H2W], f16)      # [h2 | a2src | pad]
            a2d_loc = dr.tile([TPC * P, 8], f16)        # alpha2_dst (local)
            pool_in = dr.tile([16, 512], f32)

            # resident constants
            ident = csb.tile([P, P], f32)
            make_identity(nc, ident[:])
            ident16 = csb.tile([P, P], f16)
            make_identity(nc, ident16[:])
            ones1 = csb.tile([1, P], f16)
            nc.vector.memset(ones1[:], 1.0)
            iota128 = csb.tile([P, P], f16)
            nc.sync.dma_start(out=iota128[:], in_=t_iota128[:])
            iota16 = csb.tile([P, 16], f16)
            nc.sync.dma_start(out=iota16[:], in_=t_iota16[:])
            iotacol = csb.tile([P, 1], f32)
            nc.sync.dma_start(out=iotacol[:], in_=t_iotacol[:])
            W1sb = csb.tile([P, 1024], f16)
            nc.sync.dma_start(out=W1sb[:], in_=t_W1[:])
            V1sb = csb.tile([P, 16], f16)
            nc.sync.dma_start(out=V1sb[:], in_=t_V1[:])
            W2sb = []
            V2sb = []
            for c in range(8):
                w2c = csb.tile([P, 512], f16, name=f"w2c{c}")
                nc.sync.dma_start(out=w2c[:], in_=t_W2[c * P:(c + 1) * P, :])
                W2sb.append(w2c)
                v2c = csb.tile([P, 16], f16, name=f"v2c{c}")
                nc.sync.dma_start(out=v2c[:], in_=t_V2[c * P:(c + 1) * P, :])
                V2sb.append(v2c)
            b1c = csb.tile([P, 8], f32)
            nc.sync.dma_start(out=b1c[:], in_=t_b1c[:])
            b2r = csb.tile([P, 512], f16)
            nc.sync.dma_start(out=b2r[:], in_=t_b2r[:])
            rc16 = csb.tile([16, 1], f32)
            nc.sync.dma_start(out=rc16[:], in_=t_rc16[:])
            chain_sb = csb.tile([16, 10], f32)
            nc.sync.dma_start(out=chain_sb[:], in_=t_chain[:])

            # bulk copy x into xa_tab[:, 0:128]
            nc.sync.dma_start(out=xa_tab[:, 0:P], in_=t_x16[:])

            for _rep in range(repeat):
                h2a_full = dr.tile([L2ROWS, H2W], f16, addr_space="Shared",
                                   name=f"h2a_full{_rep}")
                pool_out = dr.tile([16, 512], f32, addr_space="Shared",
                                   name=f"pool_out{_rep}")
                # ---------------- Phase A: alpha1 tables (replicated) ---------
                with (
                    tc.tile_pool(name="pa_sb", bufs=3) as asb,
                    tc.tile_pool(name="pa_ps", bufs=2, space="PSUM") as aps,
                ):
                    for t in range(NTILES_A if _inc('A') else 0):
                        sl = slice(t * P, (t + 1) * P)
                        xTt = asb.tile([P, P], f16, name="xTt")
                        nc.sync.dma_start(out=xTt[:], in_=t_xT16[:, sl])
                        al_ps = aps.tile([P, 16], f32, name="al_ps")
                        nc.tensor.matmul(al_ps[:], lhsT=xTt[:], rhs=V1sb[:],
                                         start=True, stop=True)
                        al = asb.tile([P, 16], f16, name="al")
                        nc.vector.tensor_copy(out=al[:], in_=al_ps[:])
                        nc.sync.dma_start(out=xa_tab[sl, P:P + 8], in_=al[:, 0:8])
                        nc.sync.dma_start(out=a1d_tab[sl, :], in_=al[:, 8:16])

                # ---------------- Phase B: L1 aggregation + finalize ----------
                with (
                    tc.tile_pool(name="pb_sb", bufs=4) as bsb,
                    tc.tile_pool(name="pb_big", bufs=3) as bbig,
                    tc.tile_pool(name="pb_fin", bufs=2) as bfin,
                    tc.tile_pool(name="pb_ps", bufs=1, space="PSUM") as bps,
                ):
                    for t in range(TPC if _inc('B') else 0):
                        srcm = bsb.tile([P, bt], i32, name="srcm")
                        nc.sync.dma_start(out=srcm[:], in_=t_srcm[t])
                        segm = bsb.tile([P, bt], f32, name="segm")
                        nc.sync.dma_start(out=segm[:], in_=t_segm[t])
                        segT = bsb.tile([1, bt * P], f16, name="segT")
                        nc.sync.dma_start(out=segT[:], in_=t_segT[t, None, :])
                        dstt = bsb.tile([P, 1], i32, name="dstt")
                        nc.sync.dma_start(out=dstt[:], in_=t_dstt[t])
                        aD1 = bsb.tile([P, 8], f16, name="aD1")
                        nc.gpsimd.indirect_dma_start(
                            out=aD1[:], out_offset=None, in_=a1d_tab[:],
                            in_offset=IOA(ap=dstt[:, 0:1], axis=0))

                        axden = bps.tile([P, 1536], f32, name="axden", bufs=1)
                        for b in range(bt):
                            xag = bbig.tile([P, XAW], f16, name="xag")
                            nc.gpsimd.indirect_dma_start(
                                out=xag[:], out_offset=None, in_=xa_tab[:],
                                in_offset=IOA(ap=srcm[:, b:b + 1], axis=0))
                            # ohT[s,p] = (seg[p]==s): broadcast seg along
                            # partitions via rank-1 matmul, then compare
                            sged = bps.tile([P, 136], f32, name="sged",
                                            tag="sged", bufs=1)
                            nc.tensor.matmul(sged[:, 0:P], lhsT=ones1[:],
                                             rhs=segT[0:1, b * P:(b + 1) * P],
                                             start=True, stop=True)
                            ohT = bbig.tile([P, P], f16, name="ohT")
                            nc.vector.tensor_scalar(
                                out=ohT[:], in0=sged[:, 0:P], scalar1=iotacol[:, 0:1],
                                scalar2=None, op0=OP.is_equal)
                            nc.tensor.matmul(sged[:, P:P + 8], lhsT=ohT[:],
                                             rhs=aD1[:], start=True, stop=True)
                            oh = bbig.tile([P, P], f16, name="oh")
                            nc.vector.tensor_scalar(
                                out=oh[:], in0=iota128[:], scalar1=segm[:, b:b + 1],
                                scalar2=None, op0=OP.is_equal)
                            e = bsb.tile([P, 8], f16, name="e")
                            nc.vector.tensor_tensor(out=e[:], in0=sged[:, P:P + 8],
                                                    in1=xag[:, P:P + 8], op=OP.add)
                            es = bsb.tile([P, 8], f16, name="es")
                            nc.vector.tensor_scalar_mul(out=es[:], in0=e[:],
                                                        scalar1=NEG)
                            lr = bsb.tile([P, 8], f16, name="lr")
                            nc.vector.tensor_max(out=lr[:], in0=e[:], in1=es[:])
                            ex = bsb.tile([P, 8], f16, name="ex")
                            nc.scalar.activation(ex[:], lr[:], AF.Exp)
                            msg = bbig.tile([P, 1024], f16, name="msg")
                            msg_v = msg[:].rearrange("p (h c) -> p h c", h=8)
                            xg_b = xag[:, 0:P].unsqueeze(1).broadcast_to([P, 8, P])
                            ex_b = ex[:].unsqueeze(2).broadcast_to([P, 8, P])
                            nc.vector.tensor_tensor(out=msg_v, in0=xg_b, in1=ex_b,
                                                    op=OP.mult)
                            st = (b == 0)
                            sp = (b == bt - 1)
                            nc.tensor.matmul(axden[:, 0:512], lhsT=oh[:],
                                             rhs=msg[:, 0:512], start=st, stop=sp)
                            nc.tensor.matmul(axden[:, 512:1024], lhsT=oh[:],
                                             rhs=msg[:, 512:1024], start=st, stop=sp)
                            nc.tensor.matmul(axden[:, 1024:1032], lhsT=oh[:],
                                             rhs=ex[:], start=st, stop=sp)

                        # ---- finalize tile t
                        den = bsb.tile([P, 8], f32, name="den")
                        nc.vector.tensor_scalar_max(out=den[:],
                                                    in0=axden[:, 1024:1032],
                                                    scalar1=1e-30)
                        rec = bsb.tile([P, 8], f32, name="rec")
                        nc.vector.reciprocal(out=rec[:], in_=den[:])
                        # normalize in normal space (per-partition scalar)
                        axn = bfin.tile([P, 1024], f16, name="axn")
                        for h in range(8):
                            hs = slice(h * P, (h + 1) * P)
                            nc.vector.tensor_scalar_mul(out=axn[:, hs],
                                                        in0=axden[:, hs],
                                                        scalar1=rec[:, h:h + 1])
                        h2p_ps = bps.tile([P, 512], f32, name="h2p_ps", bufs=1)
                        a2_ps = bps.tile([P, 16], f32, name="a2_ps", bufs=1)
                        for h in range(8):
                            hs = slice(h * P, (h + 1) * P)
                            tps = bps.tile([P, P], f16, name="tps", tag="tp", bufs=1)
                            nc.tensor.transpose(out=tps[:], in_=axn[:, hs],
                                                identity=ident16[:])
                            tsb = bfin.tile([P, P], f16, name="tsb", tag="ts", bufs=3)
                            nc.vector.tensor_copy(out=tsb[:], in_=tps[:])
                            # yT_h = W1_h^T @ axnT_h  [f_out, s]
                            yT = bps.tile([P, P], f32, name="yT", tag="yt", bufs=1)
                            nc.tensor.matmul(yT[:], lhsT=W1sb[:, hs], rhs=tsb[:],
                                             start=True, stop=True)
                            pre = bfin.tile([P, P], f16, name="pre", tag="pr", bufs=2)
                            nc.vector.tensor_scalar_add(out=pre[:], in0=yT[:],
                                                        scalar1=b1c[:, h:h + 1])
                            # elu in transposed space
                            m0 = bfin.tile([P, P], f16, name="m0", tag="m0", bufs=2)
                            nc.vector.tensor_scalar_min(out=m0[:], in0=pre[:],
                                                        scalar1=0.0)
                            en = bfin.tile([P, P], f16, name="en", tag="en", bufs=2)
                            nc.scalar.activation(en[:], m0[:], AF.Exp)
                            pm1 = bfin.tile([P, P], f16, name="pm1", tag="pm", bufs=2)
                            nc.vector.tensor_scalar(out=pm1[:], in0=pre[:],
                                                    scalar1=0.0, scalar2=-1.0,
                                                    op0=OP.max, op1=OP.add)
                            e1T = bfin.tile([P, P], f16, name="e1T", tag="e1", bufs=3)
                            nc.vector.tensor_add(out=e1T[:], in0=pm1[:], in1=en[:])
                            nc.tensor.matmul(h2p_ps[:], lhsT=e1T[:], rhs=W2sb[h][:],
                                             start=(h == 0), stop=(h == 7))
                            nc.tensor.matmul(a2_ps[:], lhsT=e1T[:], rhs=V2sb[h][:],
                                             start=(h == 0), stop=(h == 7))
                        pk = bfin.tile([P, H2W], f16, name="pk")
                        nc.vector.tensor_copy(out=pk[:, 0:512], in_=h2p_ps[:])
                        nc.vector.tensor_copy(out=pk[:, 512:528],
                                              in_=a2_ps[:])
                        a2d = bsb.tile([P, 8], f16, name="a2d")
                        nc.vector.tensor_copy(out=a2d[:], in_=a2_ps[:, 8:16])
                        sl = slice(t * P, (t + 1) * P)
                        nc.sync.dma_start(out=h2a_loc[sl, :], in_=pk[:])
                        nc.sync.dma_start(out=a2d_loc[sl, :], in_=a2d[:])

                # ---------------- AllGather exchange --------------------------
                if _inc('AG'):
                    nc.gpsimd.collective_compute(
                        "AllGather", mybir.AluOpType.bypass,
                        replica_groups=[list(range(NCORES))],
                        ins=[h2a_loc[:].opt()], outs=[h2a_full[:].opt()])

                # ---------------- Phase E: L2 aggregation + pool --------------
                with (
                    tc.tile_pool(name="pe_sb", bufs=4) as esb,
                    tc.tile_pool(name="pe_big", bufs=3) as ebig,
                    tc.tile_pool(name="pe_fin", bufs=2) as efin,
                    tc.tile_pool(name="pe_ps", bufs=1, space="PSUM") as eps,
                ):
                    pool_ps = eps.tile([16, 512], f32, name="pool_ps", bufs=1)
                    for t in range(TPC if _inc('E') else 0):
                        srcm = esb.tile([P, bt], i32, name="srcm2")
                        nc.sync.dma_start(out=srcm[:], in_=t_srcl2[t])
                        segm = esb.tile([P, bt], f32, name="segm2")
                        nc.sync.dma_start(out=segm[:], in_=t_segm[t])
                        segT = esb.tile([1, bt * P], f16, name="segT2")
                        nc.sync.dma_start(out=segT[:], in_=t_segT[t, None, :])
                        aD2 = esb.tile([P, 8], f16, name="aD2")
                        sl = slice(t * P, (t + 1) * P)
                        nc.sync.dma_start(out=aD2[:], in_=a2d_loc[sl, :])

                        ahden = eps.tile([P, 520], f32, name="ahden", bufs=2)
                        for b in range(bt):
                            hg = ebig.tile([P, H2W], f16, name="hg")
                            nc.gpsimd.indirect_dma_start(
                                out=hg[:], out_offset=None, in_=h2a_full[:],
                                in_offset=IOA(ap=srcm[:, b:b + 1], axis=0))
                            sged = eps.tile([P, 136], f32, name="sged2",
                                            tag="sged2", bufs=1)
                            nc.tensor.matmul(sged[:, 0:P], lhsT=ones1[:],
                                             rhs=segT[0:1, b * P:(b + 1) * P],
                                             start=True, stop=True)
                            ohT = ebig.tile([P, P], f16, name="ohT2")
                            nc.vector.tensor_scalar(
                                out=ohT[:], in0=sged[:, 0:P], scalar1=iotacol[:, 0:1],
                                scalar2=None, op0=OP.is_equal)
                            nc.tensor.matmul(sged[:, P:P + 8], lhsT=ohT[:],
                                             rhs=aD2[:], start=True, stop=True)
                            oh = ebig.tile([P, P], f16, name="oh2")
                            nc.vector.tensor_scalar(
                                out=oh[:], in0=iota128[:], scalar1=segm[:, b:b + 1],
                                scalar2=None, op0=OP.is_equal)
                            e = esb.tile([P, 8], f16, name="e2")
                            nc.vector.tensor_tensor(out=e[:], in0=sged[:, P:P + 8],
                                                    in1=hg[:, 512:520], op=OP.add)
                            es = esb.tile([P, 8], f16, name="es2")
                            nc.vector.tensor_scalar_mul(out=es[:], in0=e[:],
                                                        scalar1=NEG)
                            lr = esb.tile([P, 8], f16, name="lr2")
                            nc.vector.tensor_max(out=lr[:], in0=e[:], in1=es[:])
                            ex = esb.tile([P, 8], f16, name="ex2")
                            nc.scalar.activation(ex[:], lr[:], AF.Exp)
                            msg = ebig.tile([P, 512], f16, name="msg2")
                            msg_v = msg[:].rearrange("p (h c) -> p h c", h=8)
                            hg_v = hg[:, 0:512].rearrange("p (h c) -> p h c", h=8)
                            ex_b = ex[:].unsqueeze(2).broadcast_to([P, 8, 64])
                            nc.vector.tensor_tensor(out=msg_v, in0=hg_v, in1=ex_b,
                                                    op=OP.mult)
                            st = (b == 0)
                            sp = (b == bt - 1)
                            nc.tensor.matmul(ahden[:, 0:512], lhsT=oh[:],
                                             rhs=msg[:], start=st, stop=sp)
                            nc.tensor.matmul(ahden[:, 512:520], lhsT=oh[:],
                                             rhs=ex[:], start=st, stop=sp)

                        # ---- finalize tile t
                        den = esb.tile([P, 8], f32, name="den2")
                        nc.vector.tensor_scalar_max(out=den[:],
                                                    in0=ahden[:, 512:520],
                                                    scalar1=1e-30)
                        rec = esb.tile([P, 8], f32, name="rec2")
                        nc.vector.reciprocal(out=rec[:], in_=den[:])
                        y = efin.tile([P, 512], f16, name="yl2")
                        for h in range(8):
                            hs = slice(h * 64, (h + 1) * 64)
                            nc.vector.tensor_scalar_mul(out=y[:, hs],
                                                        in0=ahden[:, hs],
                                                        scalar1=rec[:, h:h + 1])
                        y2 = efin.tile([P, 512], f16, name="y2l2")
                        nc.vector.tensor_add(out=y2[:], in0=y[:], in1=b2r[:])
                        neg = efin.tile([P, 512], f16, name="negl2")
                        nc.vector.tensor_scalar_min(out=neg[:], in0=y2[:], scalar1=0.0)
                        en = efin.tile([P, 512], f16, name="enl2")
                        nc.scalar.activation(en[:], neg[:], AF.Exp)
                        pm1 = efin.tile([P, 512], f16, name="pm1l2")
                        nc.vector.tensor_scalar(out=pm1[:], in0=y2[:], scalar1=0.0,
                                                scalar2=-1.0, op0=OP.max, op1=OP.add)
                        e2t = efin.tile([P, 512], f16, name="e2t")
                        nc.vector.tensor_add(out=e2t[:], in0=pm1[:], in1=en[:])

                        gidt = esb.tile([P, 1], f32, name="gidt")
                        nc.sync.dma_start(out=gidt[:], in_=t_gid[t, :, None])
                        gone = esb.tile([P, 16], f16, name="gone")
                        nc.vector.tensor_scalar(out=gone[:], in0=iota16[:],
                                                scalar1=gidt[:, 0:1], scalar2=None,
                                                op0=OP.is_equal)
                        nc.tensor.matmul(pool_ps[:], lhsT=gone[:], rhs=e2t[:],
                                         start=(t == 0), stop=(t == TPC - 1))

                    # pool -> dram bounce
                    if _inc('E'):
                        pool_sb = esb.tile([16, 512], f32, name="pool_sb")
                        nc.vector.tensor_copy(out=pool_sb[:], in_=pool_ps[:])
                        nc.sync.dma_start(out=pool_in[:], in_=pool_sb[:])

                if _inc('AR'):
                    nc.gpsimd.collective_compute(
                        "AllReduce", mybir.AluOpType.add,
                        replica_groups=[list(range(NCORES))],
                        ins=[pool_in[:].opt()], outs=[pool_out[:].opt()])

                if DEBUG_DUMPS:
                    nc.sync.dma_start(out=dbg['xa'][:], in_=xa_tab[:])
                    nc.sync.dma_start(out=dbg['a1d'][:], in_=a1d_tab[:])
                    nc.sync.dma_start(out=dbg['h2a'][:], in_=h2a_loc[:])
                    nc.sync.dma_start(out=dbg['a2d'][:], in_=a2d_loc[:])
                    nc.sync.dma_start(out=dbg['pool'][:], in_=pool_in[:])

            # ---------------- Phase F: MLP (replicated) -------------------
            if not _inc('F'):
                nc.sync.dma_start(out=t_out[:], in_=chain_sb[:])
            if _inc('F'):
             with (
                tc.tile_pool(name="pf_sb", bufs=1) as fsb,
                tc.tile_pool(name="pf_ps", bufs=1, space="PSUM") as fps,
             ):
                psb = fsb.tile([16, 512], f32, name="psb")
                nc.sync.dma_start(out=psb[:], in_=pool_out[:])
                gt = fsb.tile([16, 512], f32, name="gt")
                nc.vector.tensor_scalar_mul(out=gt[:], in0=psb[:],
                                            scalar1=rc16[:, 0:1])
                fc1c = []
                for c in range(4):
                    fw = fsb.tile([P, 32], f32, name=f"fc1c{c}")
                    nc.sync.dma_start(out=fw[:], in_=t_fc1w[c * P:(c + 1) * P, :])
                    fc1c.append(fw)
                fb1 = fsb.tile([32, 1], f32, name="fb1")
                nc.sync.dma_start(out=fb1[:], in_=t_fc1b[:])
                fw2 = fsb.tile([32, 10], f32, name="fw2")
                nc.sync.dma_start(out=fw2[:], in_=t_fc2w[:])
                fb2 = fsb.tile([16, 10], f32, name="fb2")
                nc.sync.dma_start(out=fb2[:], in_=t_fc2br[:])

                fc1_ps = fps.tile([32, 16], f32, name="fc1_ps")
                for c in range(4):
                    gtt_ps = fps.tile([P, 16], f32, name="gtt_ps", tag="gtt")
                    nc.tensor.transpose(out=gtt_ps[:], in_=gt[:, c * P:(c + 1) * P],
                                        identity=ident[0:16, 0:16])
                    gtt = fsb.tile([P, 16], f32, name="gtt_sb", tag="gtts")
                    nc.vector.tensor_copy(out=gtt[:], in_=gtt_ps[:])
                    nc.tensor.matmul(fc1_ps[:], lhsT=fc1c[c][:],
                                     rhs=gtt[:],
                                     start=(c == 0), stop=(c == 3))
                y1 = fsb.tile([32, 16], f32, name="y1")
                nc.vector.tensor_scalar_add(out=y1[:], in0=fc1_ps[:],
                                            scalar1=fb1[:, 0:1])
                neg1 = fsb.tile([32, 16], f32, name="neg1")
                nc.vector.tensor_scalar_min(out=neg1[:], in0=y1[:], scalar1=0.0)
                en1 = fsb.tile([32, 16], f32, name="en1")
                nc.scalar.activation(en1[:], neg1[:], AF.Exp)
                pm11 = fsb.tile([32, 16], f32, name="pm11")
                nc.vector.tensor_scalar(out=pm11[:], in0=y1[:], scalar1=0.0,
                                        scalar2=-1.0, op0=OP.max, op1=OP.add)
                g2 = fsb.tile([32, 16], f32, name="g2")
                nc.vector.tensor_add(out=g2[:], in0=pm11[:], in1=en1[:])

                fc2_ps = fps.tile([16, 10], f32, name="fc2_ps")
                nc.tensor.matmul(fc2_ps[:], lhsT=g2[:], rhs=fw2[:],
                                 start=True, stop=True)
                osb = fsb.tile([16, 10], f32, name="osb")
                nc.vector.tensor_add(out=osb[:], in0=fc2_ps[:], in1=fb2[:])
                nc.sync.dma_start(out=t_out[:], in_=osb[:])

    nc.compile()
    return nc


def kernel(x, edge_index, batch, W1, att_src1, att_dst1, b1,
           W2, att_src2, att_dst2, b2, fc1_w, fc1_b, fc2_w, fc2_b,
           _trace=False):
    from concourse.bass_utils import run_bass_kernel_spmd
    if _trace:
        try:
            import profile_util
            profile_util.install()
        except Exception:
            pass

    x = np.asarray(x, np.float32)
    W1 = np.asarray(W1, np.float32)
    W2 = np.asarray(W2, np.float32)
    a_s1 = np.asarray(att_src1, np.float32)
    a_d1 = np.asarray(att_dst1, np.float32)
    a_s2 = np.asarray(att_src2, np.float32)
    a_d2 = np.asarray(att_dst2, np.float32)

    pp = _preprocess(np.asarray(edge_index), np.asarray(batch))
    bt = pp['bt']

    if bt not in _PROGRAM_CACHE:
        _PROGRAM_CACHE[bt] = _build_program(bt)
    nc = _PROGRAM_CACHE[bt]

    x_pad = np.zeros((NPAD, P), np.float32)
    x_pad[:N] = x
    x16 = x_pad.astype(np.float16)
    xT16 = np.ascontiguousarray(x16.T)
    V1 = np.zeros((P, 16), np.float32)
    V2 = np.zeros((1024, 16), np.float32)
    for h in range(8):
        V1[:, h] = W1[:, h * P:(h + 1) * P] @ a_s1[h]
        V1[:, 8 + h] = W1[:, h * P:(h + 1) * P] @ a_d1[h]
        V2[:, h] = W2[:, h * 64:(h + 1) * 64] @ a_s2[h]
        V2[:, 8 + h] = W2[:, h * 64:(h + 1) * 64] @ a_d2[h]

    b1c = np.asarray(b1, np.float32).reshape(8, P).T.copy()

    common = {
        "x16": x16,
        "xT16": xT16,
        "W1_16": W1.astype(np.float16),
        "V1_16": V1.astype(np.float16),
        "W2_16": W2.astype(np.float16),
        "V2_16": V2.astype(np.float16),
        "b1cols": b1c,
        "b2rep16": np.tile(np.asarray(b2, np.float16)[None, :], (P, 1)),
        "iota128_16": np.tile(np.arange(P, dtype=np.float16)[None, :], (P, 1)),
        "iota16_16": np.tile(np.arange(16, dtype=np.float16)[None, :], (P, 1)),
        "iotacol": np.arange(P, dtype=np.float32).reshape(P, 1),
        "recip_cnt16": pp['recip_cnt16'],
        "fc1_w": np.asarray(fc1_w, np.float32),
        "fc1_b": np.asarray(fc1_b, np.float32).reshape(32, 1),
        "fc2_w": np.asarray(fc2_w, np.float32),
        "fc2_b_rep": np.tile(np.asarray(fc2_b, np.float32)[None, :], (16, 1)),
    }
    in_maps = []
    for c in range(NCORES):
        m = dict(common)
        m["src_m"] = pp['src_m'][c]
        m["seg_m"] = pp['seg_m'][c]
        m["segT_m"] = pp['segT_m'][c]
        m["srcl2_m"] = pp['srcl2_m'][c]
        m["dsttile"] = pp['dsttile'][c]
        m["gid_m"] = pp['gid'][c]
        m["chain"] = np.zeros((16, 10), np.float32)
        in_maps.append(m)

    res = run_bass_kernel_spmd(nc, in_maps, list(range(NCORES)),
                               trace=bool(_trace))
    LAST_PROFILE.clear()
    LAST_PROFILE['exec_time_ns'] = res.exec_time_ns
    LAST_PROFILE['results'] = res
    return np.asarray(res.results[0]["out"], np.float32)


# revision 10
# speedup vs baseline: 1.7963x; 1.3282x over previous
"""GAT (2x GATConv + global_mean_pool + MLP) on 8 Trainium2 NeuronCores.

Strategy (sharding_hint: 1D node partition, replicated weights):
  - dst nodes partitioned 8 ways (1250/core, 10 tiles of 128 slots);
    edges sorted by dst, packed into per-(core,tile) blocks of 128.
  - Layer 1 aggregates x[src] (128 wide) instead of h[src] (1024 wide):
    sum_e ex*(x W1) == (sum_e ex*x) W1 per head -> 8x less gather traffic.
    Attention logits via fused vectors V = W @ a (alpha = x @ V).
  - All aggregation math in fp16 (PE 4x faster than f32; gathers half the
    bytes); PSUM accumulation stays f32.
  - alpha_dst is NOT gathered per edge: dst rows of a tile are contiguous,
    so fetch the tile's [128,8] alphas once and permute per block with a
    one-hot matmul (ohT built from a host-side transposed segment table
    via a rank-1 broadcast matmul + is_equal, batched 4 blocks at a time).
  - Vector/scalar engines split the per-head msg multiplies and
    normalization (4 heads each) to balance load.
  - Only exchange: AllGather of per-core [1280, 528] fp16 packed table
    (h2 = elu(out1) @ W2 | alpha2_src | alpha2_dst), done in 2 halves so
    the first overlaps the second half of layer-1 compute; plus a tiny
    AllReduce of pooled per-graph sums. MLP replicated.
"""
import os
import sys
import numpy as np

for _p in ("/opt/trn_rl_repo",):
    if os.path.isdir(_p) and _p not in sys.path:
        sys.path.insert(0, _p)

N = 10000
B = 16
NCORES = 8
P = 128
NPC = 1250                  # nodes per core
TPC = 10                    # dst tiles per core
NPAD = 10112                # 79 * 128
NTILES_A = 79
NEG = 0.2
L2ROWS = NCORES * TPC * P   # 10240
XAW = 144                   # xa row: 128 x | 8 a_src | 8 a_dst (f16)
H2W = 528                   # h2a row: 512 h2 | 8 a2src | 8 a2dst (f16)
HROWS = TPC * P // 2        # 640 rows per AllGather half
G = 3                       # blocks per inner group

_PROGRAM_CACHE = {}
LAST_PROFILE = {}
DEBUG_DUMPS = False


def _preprocess(edge_index, batch):
    src = np.concatenate([np.asarray(edge_index[0]), np.arange(N)]).astype(np.int64)
    dst = np.concatenate([np.asarray(edge_index[1]), np.arange(N)]).astype(np.int64)
    order = np.argsort(dst, kind='stable')
    src, dst = src[order], dst[order]

    core_of = dst // NPC
    local = dst - core_of * NPC
    tile_of = local // P
    seg_of = (local - tile_of * P).astype(np.float32)

    counts = np.zeros((NCORES, TPC), dtype=np.int64)
    np.add.at(counts, (core_of, tile_of), 1)
    bt = int(np.ceil(counts.max() / P))
    bt = max(bt, 1)

    src_m = np.zeros((NCORES, TPC, P, bt), dtype=np.int32)
    seg_m = np.full((NCORES, TPC, P, bt), -1.0, dtype=np.float32)

    flat_group = core_of * TPC + tile_of
    grp_start = np.searchsorted(flat_group, np.arange(NCORES * TPC), 'left')
    rank = np.arange(len(flat_group)) - grp_start[flat_group]
    blk = rank // P
    part = rank % P
    co = core_of.astype(np.int64)
    ti = tile_of.astype(np.int64)
    src_m[co, ti, part, blk] = src.astype(np.int32)
    seg_m[co, ti, part, blk] = seg_of

    # transposed segment table: segT[c, t, b*128 + j] = seg_m[c, t, j, b]
    segT_m = np.ascontiguousarray(
        seg_m.transpose(0, 1, 3, 2).reshape(NCORES, TPC, bt * P)).astype(np.float16)

    # L2 rows, split into two half-tables for the 2-phase AllGather:
    # node n on core c, local r in [0,1280): half = r//640,
    # row within its half-table = c*640 + (r - half*640)
    node = np.arange(N, dtype=np.int64)
    cn = node // NPC
    r = node - cn * NPC
    half_of = (r // HROWS).astype(np.int64)
    l2half = (cn * HROWS + (r - half_of * HROWS)).astype(np.int32)

    # partition each (core,tile)'s edges by source half; repack blocks
    srcA = [[None] * TPC for _ in range(NCORES)]
    btA = btB = 1
    packs = []
    for c in range(NCORES):
        for t in range(TPC):
            m = (co == c) & (ti == t)
            s_, g_ = src[m], seg_of[m]
            hmask = half_of[s_] == 0
            packs.append((c, t, s_[hmask], g_[hmask], s_[~hmask], g_[~hmask]))
            btA = max(btA, int(np.ceil(hmask.sum() / P)))
            btB = max(btB, int(np.ceil((~hmask).sum() / P)))

    def _pack(npart, s_, g_):
        sm = np.zeros((P, npart), dtype=np.int32)
        gm = np.full((P, npart), -1.0, dtype=np.float32)
        k = np.arange(len(s_))
        sm[k % P, k // P] = l2half[s_]
        gm[k % P, k // P] = g_
        return sm, gm

    srcA_m = np.zeros((NCORES, TPC, P, btA), dtype=np.int32)
    segA_m = np.full((NCORES, TPC, P, btA), -1.0, dtype=np.float32)
    srcB_m = np.zeros((NCORES, TPC, P, btB), dtype=np.int32)
    segB_m = np.full((NCORES, TPC, P, btB), -1.0, dtype=np.float32)
    for c, t, sA, gA, sB, gB in packs:
        srcA_m[c, t], segA_m[c, t] = _pack(btA, sA, gA)
        srcB_m[c, t], segB_m[c, t] = _pack(btB, sB, gB)
    segTA_m = np.ascontiguousarray(
        segA_m.transpose(0, 1, 3, 2).reshape(NCORES, TPC, btA * P)).astype(np.float16)
    segTB_m = np.ascontiguousarray(
        segB_m.transpose(0, 1, 3, 2).reshape(NCORES, TPC, btB * P)).astype(np.float16)

    # per-tile dst node ids (rows of xa_tab): core c, tile t, slot p
    dsttile = np.zeros((NCORES, TPC, P, 1), dtype=np.int32)
    for c in range(NCORES):
        for t in range(TPC):
            ids = c * NPC + t * P + np.arange(P)
            ids = np.minimum(ids, (c + 1) * NPC - 1)   # clamp pad slots
            dsttile[c, t, :, 0] = ids

    batch = np.asarray(batch).astype(np.int64)
    gid = np.full((NCORES, TPC, P), -1.0, dtype=np.float32)
    for c in range(NCORES):
        lo = c * NPC
        hi = min(lo + NPC, N)
        vals = batch[lo:hi].astype(np.float32)
        g = gid[c].reshape(-1)
        g[:hi - lo] = vals

    cnt = np.zeros(B, np.float32)
    np.add.at(cnt, batch, 1.0)
    recip_cnt16 = (1.0 / np.maximum(cnt, 1.0)).astype(np.float32).reshape(16, 1)

    return dict(bt=bt, btA=btA, btB=btB, src_m=src_m, seg_m=seg_m,
                segT_m=segT_m, srcA_m=srcA_m, segA_m=segA_m, segTA_m=segTA_m,
                srcB_m=srcB_m, segB_m=segB_m, segTB_m=segTB_m,
                dsttile=dsttile, gid=gid, recip_cnt16=recip_cnt16)


def _build_program(bt, btA, btB, upto='full', repeat=1):
    import concourse.bacc as bacc
    import concourse.bass as bass
    import concourse.mybir as mybir
    import concourse.tile as tile
    from concourse.masks import make_identity

    f32 = mybir.dt.float32
    f16 = mybir.dt.float16
    i32 = mybir.dt.int32
    AF = mybir.ActivationFunctionType
    OP = mybir.AluOpType
    IOA = bass.IndirectOffsetOnAxis

    _ORDER = ['none', 'A', 'B', 'AG', 'E', 'AR', 'F', 'full']
    def _inc(s):
        return _ORDER.index(upto if upto != 'full' else 'F') >= _ORDER.index(s)

    nc = bacc.Bacc("TRN2", target_bir_lowering=False, debug=False,
                   enable_asserts=False, num_devices=NCORES)

    # ---------------- inputs ----------------
    t_x16 = nc.dram_tensor("x16", [NPAD, P], f16, kind="ExternalInput")
    t_xT16 = nc.dram_tensor("xT16", [P, NPAD], f16, kind="ExternalInput")
    t_W1 = nc.dram_tensor("W1_16", [P, 1024], f16, kind="ExternalInput")
    t_V1 = nc.dram_tensor("V1_16", [P, 16], f16, kind="ExternalInput")
    t_W2 = nc.dram_tensor("W2_16", [1024, 512], f16, kind="ExternalInput")
    t_V2 = nc.dram_tensor("V2_16", [1024, 16], f16, kind="ExternalInput")
    t_b1c = nc.dram_tensor("b1cols", [P, 8], f32, kind="ExternalInput")
    t_b2r = nc.dram_tensor("b2rep16", [P, 512], f16, kind="ExternalInput")
    t_iota128 = nc.dram_tensor("iota128_16", [P, P], f16, kind="ExternalInput")
    t_iota16 = nc.dram_tensor("iota16_16", [P, 16], f16, kind="ExternalInput")
    t_iotacol = nc.dram_tensor("iotacol", [P, 1], f32, kind="ExternalInput")
    t_rc16 = nc.dram_tensor("recip_cnt16", [16, 1], f32, kind="ExternalInput")
    t_fc1w = nc.dram_tensor("fc1_w", [512, 32], f32, kind="ExternalInput")
    t_fc1b = nc.dram_tensor("fc1_b", [32, 1], f32, kind="ExternalInput")
    t_fc2w = nc.dram_tensor("fc2_w", [32, 10], f32, kind="ExternalInput")
    t_fc2br = nc.dram_tensor("fc2_b_rep", [16, 10], f32, kind="ExternalInput")
    t_srcm = nc.dram_tensor("src_m", [TPC, P, bt], i32, kind="ExternalInput")
    t_segm = nc.dram_tensor("seg_m", [TPC, P, bt], f32, kind="ExternalInput")
    t_segT = nc.dram_tensor("segT_m", [TPC, bt * P], f16, kind="ExternalInput")
    t_srcA = nc.dram_tensor("srcA_m", [TPC, P, btA], i32, kind="ExternalInput")
    t_segA = nc.dram_tensor("segA_m", [TPC, P, btA], f32, kind="ExternalInput")
    t_segTA = nc.dram_tensor("segTA_m", [TPC, btA * P], f16, kind="ExternalInput")
    t_srcB = nc.dram_tensor("srcB_m", [TPC, P, btB], i32, kind="ExternalInput")
    t_segB = nc.dram_tensor("segB_m", [TPC, P, btB], f32, kind="ExternalInput")
    t_segTB = nc.dram_tensor("segTB_m", [TPC, btB * P], f16, kind="ExternalInput")
    t_dstt = nc.dram_tensor("dsttile", [TPC, P, 1], i32, kind="ExternalInput")
    t_gid = nc.dram_tensor("gid_m", [TPC, P], f32, kind="ExternalInput")
    t_chain = nc.dram_tensor("chain", [16, 10], f32, kind="ExternalInput")

    t_out = nc.dram_tensor("out", [16, 10], f32, kind="ExternalOutput")
    dbg = {}
    if DEBUG_DUMPS:
        dbg['xa'] = nc.dram_tensor("dbg_xa", [NPAD, XAW], f16, kind="ExternalOutput")
        dbg['h2a'] = nc.dram_tensor("dbg_h2a", [TPC * P, H2W], f16, kind="ExternalOutput")
        dbg['pool'] = nc.dram_tensor("dbg_pool", [16, 512], f32, kind="ExternalOutput")

    def groups(n):
        out, b = [], 0
        while b < n:
            out.append(list(range(b, min(b + G, n))))
            b += G
        return out

    with tile.TileContext(nc) as tc:
        with (
            tc.tile_pool(name="const", bufs=1) as csb,
            tc.tile_pool(name="dram", bufs=1, space="DRAM") as dr,
        ):
            # DRAM staging
            xa_tab = dr.tile([NPAD, XAW], f16)          # [x | a_src | a_dst]
            h2a_loc = dr.tile([TPC * P, H2W], f16)      # [h2 | a2src | a2dst]
            pool_in = dr.tile([16, 512], f32)

            # resident constants
            ident = csb.tile([P, P], f32)
            make_identity(nc, ident[:])
            ident16 = csb.tile([P, P], f16)
            make_identity(nc, ident16[:])
            ones1 = csb.tile([1, P], f16)
            nc.vector.memset(ones1[:], 1.0)
            iota128 = csb.tile([P, P], f16)
            nc.sync.dma_start(out=iota128[:], in_=t_iota128[:])
            iota16 = csb.tile([P, 16], f16)
            nc.sync.dma_start(out=iota16[:], in_=t_iota16[:])
            iotacol = csb.tile([P, 1], f32)
            nc.sync.dma_start(out=iotacol[:], in_=t_iotacol[:])
            W1sb = csb.tile([P, 1024], f16)
            nc.sync.dma_start(out=W1sb[:], in_=t_W1[:])
            V1sb = csb.tile([P, 16], f16)
            nc.sync.dma_start(out=V1sb[:], in_=t_V1[:])
            W2sb = []
            V2sb = []
            for c in range(8):
                w2c = csb.tile([P, 512], f16, name=f"w2c{c}")
                nc.sync.dma_start(out=w2c[:], in_=t_W2[c * P:(c + 1) * P, :])
                W2sb.append(w2c)
                v2c = csb.tile([P, 16], f16, name=f"v2c{c}")
                nc.sync.dma_start(out=v2c[:], in_=t_V2[c * P:(c + 1) * P, :])
                V2sb.append(v2c)
            b1c = csb.tile([P, 8], f32)
            nc.sync.dma_start(out=b1c[:], in_=t_b1c[:])
            b2r = csb.tile([P, 512], f16)
            nc.sync.dma_start(out=b2r[:], in_=t_b2r[:])
            rc16 = csb.tile([16, 1], f32)
            nc.sync.dma_start(out=rc16[:], in_=t_rc16[:])
            chain_sb = csb.tile([16, 10], f32)
            nc.sync.dma_start(out=chain_sb[:], in_=t_chain[:])

            for _rep in range(repeat):
                h2a_A = dr.tile([NCORES * HROWS, H2W], f16, addr_space="Shared",
                                name=f"h2a_A{_rep}")
                h2a_B = dr.tile([NCORES * HROWS, H2W], f16, addr_space="Shared",
                                name=f"h2a_B{_rep}")
                pool_out = dr.tile([16, 512], f32, addr_space="Shared",
                                   name=f"pool_out{_rep}")
                # ---------------- Phase A: xa table (replicated) --------------
                with (
                    tc.tile_pool(name="pa_xt", bufs=1) as axt,
                    tc.tile_pool(name="pa_sb", bufs=4) as asb,
                    tc.tile_pool(name="pa_ps", bufs=3, space="PSUM") as aps,
                ):
                    xTsb = axt.tile([P, NPAD], f16, name="xTsb")
                    nc.sync.dma_start(out=xTsb[:], in_=t_xT16[:])
                    for t in range(NTILES_A if _inc('A') else 0):
                        sl = slice(t * P, (t + 1) * P)
                        xrow = asb.tile([P, P], f16, name="xrow")
                        nc.sync.dma_start(out=xrow[:], in_=t_x16[sl, :])
                        al_ps = aps.tile([P, 16], f32, name="al_ps")
                        nc.tensor.matmul(al_ps[:], lhsT=xTsb[:, sl], rhs=V1sb[:],
                                         start=True, stop=True)
                        mrg = asb.tile([P, XAW], f16, name="mrg")
                        nc.vector.tensor_copy(out=mrg[:, 0:P], in_=xrow[:])
                        nc.vector.tensor_copy(out=mrg[:, P:P + 16], in_=al_ps[:])
                        nc.sync.dma_start(out=xa_tab[sl, :], in_=mrg[:])

                # ---------------- Phase B: L1 aggregation + finalize ----------
                with (
                    tc.tile_pool(name="pb_sb", bufs=4) as bsb,
                    tc.tile_pool(name="pb_big", bufs=3) as bbig,
                    tc.tile_pool(name="pb_fin", bufs=2) as bfin,
                    tc.tile_pool(name="pb_ps", bufs=1, space="PSUM") as bps,
                ):
                    for t in range(TPC if _inc('B') else 0):
                        srcm = bsb.tile([P, bt], i32, name="srcm")
                        nc.sync.dma_start(out=srcm[:], in_=t_srcm[t])
                        segm = bsb.tile([P, bt], f32, name="segm")
                        nc.sync.dma_start(out=segm[:], in_=t_segm[t])
                        segT = bsb.tile([1, bt * P], f16, name="segT")
                        nc.sync.dma_start(out=segT[:], in_=t_segT[t, None, :])
                        dstt = bsb.tile([P, 1], i32, name="dstt")
                        nc.sync.dma_start(out=dstt[:], in_=t_dstt[t])
                        aD1t = bsb.tile([P, XAW], f16, name="aD1t")
                        nc.gpsimd.indirect_dma_start(
                            out=aD1t[:], out_offset=None, in_=xa_tab[:],
                            in_offset=IOA(ap=dstt[:, 0:1], axis=0))

                        # axden cols: 0:1024 num, 1024:1032 den, 1040:1072 edst
                        axden = bps.tile([P, 1536], f32, name="axden", bufs=1)
                        for grp in groups(bt):
                            g0, gsz = grp[0], len(grp)
                            xag = bbig.tile([P, G * XAW], f16, name="xag")
                            for k, b in enumerate(grp):
                                nc.gpsimd.indirect_dma_start(
                                    out=xag[:, k * XAW:(k + 1) * XAW],
                                    out_offset=None, in_=xa_tab[:],
                                    in_offset=IOA(ap=srcm[:, b:b + 1], axis=0))
                            sged = bps.tile([P, G * P + G * 8], f32,
                                            name="sged", tag="sged", bufs=1)
                            nc.tensor.matmul(
                                sged[:, 0:gsz * P], lhsT=ones1[:],
                                rhs=segT[0:1, g0 * P:(g0 + gsz) * P],
                                start=True, stop=True)
                            ohT = bbig.tile([P, G * P], f16, name="ohT")
                            nc.vector.tensor_scalar(
                                out=ohT[:, 0:gsz * P], in0=sged[:, 0:gsz * P],
                                scalar1=iotacol[:, 0:1],
                                scalar2=None, op0=OP.is_equal)
                            for k in range(gsz):
                                nc.tensor.matmul(
                                    sged[:, G * P + k * 8:G * P + (k + 1) * 8],
                                    lhsT=ohT[:, k * P:(k + 1) * P],
                                    rhs=aD1t[:, XAW - 8:XAW],
                                    start=True, stop=True, skip_group_check=True)
                            e4 = bsb.tile([P, G * 8], f16, name="e4")
                            asrc_v = xag[:].rearrange("p (k w) -> p k w", k=G)
                            nc.vector.tensor_tensor(
                                out=e4[:, 0:gsz * 8].rearrange("p (k w) -> p k w", k=gsz),
                                in0=sged[:, G * P:G * P + gsz * 8].rearrange(
                                    "p (k w) -> p k w", k=gsz),
                                in1=asrc_v[:, 0:gsz, P:P + 8], op=OP.add)
                            es4 = bsb.tile([P, G * 8], f16, name="es4")
                            nc.vector.tensor_scalar_mul(out=es4[:, 0:gsz * 8],
                                                        in0=e4[:, 0:gsz * 8],
                                                        scalar1=NEG)
                            lr4 = bsb.tile([P, G * 8], f16, name="lr4")
                            nc.vector.tensor_max(out=lr4[:, 0:gsz * 8],
                                                 in0=e4[:, 0:gsz * 8],
                                                 in1=es4[:, 0:gsz * 8])
                            ex4 = bsb.tile([P, G * 8], f32, name="ex4")
                            nc.scalar.activation(ex4[:, 0:gsz * 8],
                                                 lr4[:, 0:gsz * 8], AF.Exp)
                            ex16 = bsb.tile([P, G * 8], f16, name="ex16")
                            nc.vector.tensor_copy(out=ex16[:, 0:gsz * 8],
                                                  in_=ex4[:, 0:gsz * 8])

                            for k, b in enumerate(grp):
                                oh = bbig.tile([P, P], f16, name="oh")
                                nc.vector.tensor_scalar(
                                    out=oh[:], in0=iota128[:],
                                    scalar1=segm[:, b:b + 1],
                                    scalar2=None, op0=OP.is_equal)
                                msg = bbig.tile([P, 1024], f16, name="msg")
                                xs = xag[:, k * XAW:k * XAW + P]
                                for h in range(4):
                                    nc.vector.tensor_scalar_mul(
                                        out=msg[:, h * P:(h + 1) * P], in0=xs,
                                        scalar1=ex4[:, k * 8 + h:k * 8 + h + 1])
                                for h in range(4, 8):
                                    nc.scalar.activation(
                                        msg[:, h * P:(h + 1) * P], xs, AF.Copy,
                                        scale=ex4[:, k * 8 + h:k * 8 + h + 1])
                                st = (b == 0)
                                sp = (b == bt - 1)
                                nc.tensor.matmul(axden[:, 0:512], lhsT=oh[:],
                                                 rhs=msg[:, 0:512],
                                                 start=st, stop=sp)
                                nc.tensor.matmul(axden[:, 512:1024], lhsT=oh[:],
                                                 rhs=msg[:, 512:1024],
                                                 start=st, stop=sp)
                                nc.tensor.matmul(axden[:, 1024:1032], lhsT=oh[:],
                                                 rhs=ex16[:, k * 8:(k + 1) * 8],
                                                 start=st, stop=sp)

                        # ---- finalize tile t
                        den = bsb.tile([P, 8], f32, name="den")
                        nc.vector.tensor_scalar_max(out=den[:],
                                                    in0=axden[:, 1024:1032],
                                                    scalar1=1e-30)
                        rec = bsb.tile([P, 8], f32, name="rec")
                        nc.vector.reciprocal(out=rec[:], in_=den[:])
                        # normalize (4 heads DVE, 4 heads scalar)
                        axn = bfin.tile([P, 1024], f16, name="axn")
                        for h in range(4):
                            hs = slice(h * P, (h + 1) * P)
                            nc.vector.tensor_scalar_mul(out=axn[:, hs],
                                                        in0=axden[:, hs],
                                                        scalar1=rec[:, h:h + 1])
                        for h in range(4, 8):
                            hs = slice(h * P, (h + 1) * P)
                            nc.scalar.activation(axn[:, hs], axden[:, hs],
                                                 AF.Copy, scale=rec[:, h:h + 1])
                        h2p_ps = bps.tile([P, 512], f32, name="h2p_ps", bufs=1)
                        a2_ps = bps.tile([P, 16], f32, name="a2_ps", bufs=1)
                        for h in range(8):
                            hs = slice(h * P, (h + 1) * P)
                            tps = bps.tile([P, P], f16, name="tps", tag="tp", bufs=1)
                            nc.tensor.transpose(out=tps[:], in_=axn[:, hs],
                                                identity=ident16[:])
                            tsb = bfin.tile([P, P], f16, name="tsb", tag="ts", bufs=3)
                            nc.vector.tensor_copy(out=tsb[:], in_=tps[:])
                            # yT_h = W1_h^T @ axnT_h  [f_out, s]
                            yT = bps.tile([P, P], f32, name="yT", tag="yt", bufs=1)
                            nc.tensor.matmul(yT[:], lhsT=W1sb[:, hs], rhs=tsb[:],
                                             start=True, stop=True)
                            pre = bfin.tile([P, P], f16, name="pre", tag="pr", bufs=2)
                            nc.vector.tensor_scalar_add(out=pre[:], in0=yT[:],
                                                        scalar1=b1c[:, h:h + 1])
                            # elu in transposed space
                            m0 = bfin.tile([P, P], f16, name="m0", tag="m0", bufs=2)
                            nc.vector.tensor_scalar_min(out=m0[:], in0=pre[:],
                                                        scalar1=0.0)
                            en = bfin.tile([P, P], f16, name="en", tag="en", bufs=2)
                            nc.scalar.activation(en[:], m0[:], AF.Exp)
                            pm1 = bfin.tile([P, P], f16, name="pm1", tag="pm", bufs=2)
                            nc.vector.tensor_scalar(out=pm1[:], in0=pre[:],
                                                    scalar1=0.0, scalar2=-1.0,
                                                    op0=OP.max, op1=OP.add)
                            e1T = bfin.tile([P, P], f16, name="e1T", tag="e1", bufs=3)
                            nc.vector.tensor_add(out=e1T[:], in0=pm1[:], in1=en[:])
                            nc.tensor.matmul(h2p_ps[:], lhsT=e1T[:], rhs=W2sb[h][:],
                                             start=(h == 0), stop=(h == 7))
                            nc.tensor.matmul(a2_ps[:], lhsT=e1T[:], rhs=V2sb[h][:],
                                             start=(h == 0), stop=(h == 7))
                        pk = bfin.tile([P, H2W], f16, name="pk")
                        nc.vector.tensor_copy(out=pk[:, 0:512], in_=h2p_ps[:])
                        nc.vector.tensor_copy(out=pk[:, 512:528], in_=a2_ps[:])
                        sl = slice(t * P, (t + 1) * P)
                        nc.sync.dma_start(out=h2a_loc[sl, :], in_=pk[:])

                        # first-half AllGather as soon as tiles 0-4 are done
                        if _inc('AG') and t == TPC // 2 - 1:
                            nc.gpsimd.collective_compute(
                                "AllGather", mybir.AluOpType.bypass,
                                replica_groups=[list(range(NCORES))],
                                ins=[h2a_loc[0:HROWS, :].opt()],
                                outs=[h2a_A[:].opt()])

                if _inc('AG'):
                    nc.gpsimd.collective_compute(
                        "AllGather", mybir.AluOpType.bypass,
                        replica_groups=[list(range(NCORES))],
                        ins=[h2a_loc[HROWS:2 * HROWS, :].opt()],
                        outs=[h2a_B[:].opt()])

                # ---------------- Phase E: L2 aggregation + pool --------------
                with (
                    tc.tile_pool(name="pe_sb", bufs=4) as esb,
                    tc.tile_pool(name="pe_big", bufs=3) as ebig,
                    tc.tile_pool(name="pe_fin", bufs=2) as efin,
                    tc.tile_pool(name="pe_ps", bufs=1, space="PSUM") as eps,
                ):
                    pool_ps = eps.tile([16, 512], f32, name="pool_ps", bufs=1)
                    nblk = btA + btB
                    for t in range(TPC if _inc('E') else 0):
                        passes = []
                        for pn, (t_src, t_seg, t_segT, btP, htab) in enumerate(
                                [(t_srcA, t_segA, t_segTA, btA, h2a_A),
                                 (t_srcB, t_segB, t_segTB, btB, h2a_B)]):
                            srcm = esb.tile([P, btP], i32, name=f"srcm2{pn}")
                            nc.sync.dma_start(out=srcm[:], in_=t_src[t])
                            segm = esb.tile([P, btP], f32, name=f"segm2{pn}")
                            nc.sync.dma_start(out=segm[:], in_=t_seg[t])
                            segT = esb.tile([1, btP * P], f16, name=f"segT2{pn}")
                            nc.sync.dma_start(out=segT[:], in_=t_segT[t, None, :])
                            passes.append((srcm, segm, segT, btP, htab))
                        aD2 = esb.tile([P, 8], f16, name="aD2")
                        sl = slice(t * P, (t + 1) * P)
                        nc.sync.dma_start(out=aD2[:], in_=h2a_loc[sl, 520:528])

                        # ahden cols: 0:512 num, 512:520 den, 528:560 edst
                        ahden = eps.tile([P, 1024], f32, name="ahden", bufs=2)
                        bglob = 0
                        for srcm, segm, segT, btP, htab in passes:
                            for grp in groups(btP):
                                g0, gsz = grp[0], len(grp)
                                hg = ebig.tile([P, G * H2W], f16, name="hg")
                                for k, b in enumerate(grp):
                                    nc.gpsimd.indirect_dma_start(
                                        out=hg[:, k * H2W:(k + 1) * H2W],
                                        out_offset=None, in_=htab[:],
                                        in_offset=IOA(ap=srcm[:, b:b + 1], axis=0))
                                sged = eps.tile([P, G * P + G * 8], f32,
                                                name="sged2", tag="sged2", bufs=1)
                                nc.tensor.matmul(
                                    sged[:, 0:gsz * P], lhsT=ones1[:],
                                    rhs=segT[0:1, g0 * P:(g0 + gsz) * P],
                                    start=True, stop=True)
                                ohT = ebig.tile([P, G * P], f16, name="ohT2")
                                nc.vector.tensor_scalar(
                                    out=ohT[:, 0:gsz * P], in0=sged[:, 0:gsz * P],
                                    scalar1=iotacol[:, 0:1],
                                    scalar2=None, op0=OP.is_equal)
                                for k in range(gsz):
                                    nc.tensor.matmul(
                                        sged[:, G * P + k * 8:G * P + (k + 1) * 8],
                                        lhsT=ohT[:, k * P:(k + 1) * P],
                                        rhs=aD2[:], start=True, stop=True,
                                        skip_group_check=True)
                                e4 = esb.tile([P, G * 8], f16, name="e42")
                                asrc_v = hg[:].rearrange("p (k w) -> p k w", k=G)
                                nc.vector.tensor_tensor(
                                    out=e4[:, 0:gsz * 8].rearrange(
                                        "p (k w) -> p k w", k=gsz),
                                    in0=sged[:, G * P:G * P + gsz * 8].rearrange(
                                        "p (k w) -> p k w", k=gsz),
                                    in1=asrc_v[:, 0:gsz, 512:520], op=OP.add)
                                es4 = esb.tile([P, G * 8], f16, name="es42")
                                nc.vector.tensor_scalar_mul(out=es4[:, 0:gsz * 8],
                                                            in0=e4[:, 0:gsz * 8],
                                                            scalar1=NEG)
                                lr4 = esb.tile([P, G * 8], f16, name="lr42")
                                nc.vector.tensor_max(out=lr4[:, 0:gsz * 8],
                                                     in0=e4[:, 0:gsz * 8],
                                                     in1=es4[:, 0:gsz * 8])
                                ex4 = esb.tile([P, G * 8], f32, name="ex42")
                                nc.scalar.activation(ex4[:, 0:gsz * 8],
                                                     lr4[:, 0:gsz * 8], AF.Exp)
                                ex16 = esb.tile([P, G * 8], f16, name="ex162")
                                nc.vector.tensor_copy(out=ex16[:, 0:gsz * 8],
                                                      in_=ex4[:, 0:gsz * 8])

                                for k, b in enumerate(grp):
                                    oh = ebig.tile([P, P], f16, name="oh2")
                                    nc.vector.tensor_scalar(
                                        out=oh[:], in0=iota128[:],
                                        scalar1=segm[:, b:b + 1],
                                        scalar2=None, op0=OP.is_equal)
                                    msg = ebig.tile([P, 512], f16, name="msg2")
                                    hs_ = hg[:, k * H2W:k * H2W + 512]
                                    for h in range(4):
                                        nc.vector.tensor_scalar_mul(
                                            out=msg[:, h * 64:(h + 1) * 64],
                                            in0=hs_[:, h * 64:(h + 1) * 64],
                                            scalar1=ex4[:, k * 8 + h:k * 8 + h + 1])
                                    for h in range(4, 8):
                                        nc.scalar.activation(
                                            msg[:, h * 64:(h + 1) * 64],
                                            hs_[:, h * 64:(h + 1) * 64], AF.Copy,
                                            scale=ex4[:, k * 8 + h:k * 8 + h + 1])
                                    st = (bglob == 0)
                                    sp = (bglob == nblk - 1)
                                    nc.tensor.matmul(ahden[:, 0:512], lhsT=oh[:],
                                                     rhs=msg[:], start=st, stop=sp)
                                    nc.tensor.matmul(ahden[:, 512:520], lhsT=oh[:],
                                                     rhs=ex16[:, k * 8:(k + 1) * 8],
                                                     start=st, stop=sp)
                                    bglob += 1

                        # ---- finalize tile t
                        den = esb.tile([P, 8], f32, name="den2")
                        nc.vector.tensor_scalar_max(out=den[:],
                                                    in0=ahden[:, 512:520],
                                                    scalar1=1e-30)
                        rec = esb.tile([P, 8], f32, name="rec2")
                        nc.vector.reciprocal(out=rec[:], in_=den[:])
                        y = efin.tile([P, 512], f16, name="yl2")
                        for h in range(4):
                            hs = slice(h * 64, (h + 1) * 64)
                            nc.vector.tensor_scalar_mul(out=y[:, hs],
                                                        in0=ahden[:, hs],
                                                        scalar1=rec[:, h:h + 1])
                        for h in range(4, 8):
                            hs = slice(h * 64, (h + 1) * 64)
                            nc.scalar.activation(y[:, hs], ahden[:, hs],
                                                 AF.Copy, scale=rec[:, h:h + 1])
                        y2 = efin.tile([P, 512], f16, name="y2l2")
                        nc.vector.tensor_add(out=y2[:], in0=y[:], in1=b2r[:])
                        neg = efin.tile([P, 512], f16, name="negl2")
                        nc.vector.tensor_scalar_min(out=neg[:], in0=y2[:], scalar1=0.0)
                        en = efin.tile([P, 512], f16, name="enl2")
                        nc.scalar.activation(en[:], neg[:], AF.Exp)
                        pm1 = efin.tile([P, 512], f16, name="pm1l2")
                        nc.vector.tensor_scalar(out=pm1[:], in0=y2[:], scalar1=0.0,
                                                scalar2=-1.0, op0=OP.max, op1=OP.add)
                        e2t = efin.tile([P, 512], f16, name="e2t")
                        nc.vector.tensor_add(out=e2t[:], in0=pm1[:], in1=en[:])

                        gidt = esb.tile([P, 1], f32, name="gidt")
                        nc.sync.dma_start(out=gidt[:], in_=t_gid[t, :, None])
                        gone = esb.tile([P, 16], f16, name="gone")
                        nc.vector.tensor_scalar(out=gone[:], in0=iota16[:],
                                                scalar1=gidt[:, 0:1], scalar2=None,
                                                op0=OP.is_equal)
                        nc.tensor.matmul(pool_ps[:], lhsT=gone[:], rhs=e2t[:],
                                         start=(t == 0), stop=(t == TPC - 1))

                    # pool -> dram bounce
                    if _inc('E'):
                        pool_sb = esb.tile([16, 512], f32, name="pool_sb")
                        nc.vector.tensor_copy(out=pool_sb[:], in_=pool_ps[:])
                        nc.sync.dma_start(out=pool_in[:], in_=pool_sb[:])

                if _inc('AR'):
                    nc.gpsimd.collective_compute(
                        "AllReduce", mybir.AluOpType.add,
                        replica_groups=[list(range(NCORES))],
                        ins=[pool_in[:].opt()], outs=[pool_out[:].opt()])

                if DEBUG_DUMPS:
                    nc.sync.dma_start(out=dbg['xa'][:], in_=xa_tab[:])
                    nc.sync.dma_start(out=dbg['h2a'][:], in_=h2a_loc[:])
                    nc.sync.dma_start(out=dbg['pool'][:], in_=pool_in[:])

            # ---------------- Phase F: MLP (replicated) -------------------
            if not _inc('F'):
                nc.sync.dma_start(out=t_out[:], in_=chain_sb[:])
            if _inc('F'):
             with (
                tc.tile_pool(name="pf_sb", bufs=1) as fsb,
                tc.tile_pool(name="pf_ps", bufs=1, space="PSUM") as fps,
             ):
                psb = fsb.tile([16, 512], f32, name="psb")
                nc.sync.dma_start(out=psb[:], in_=pool_out[:])
                gt = fsb.tile([16, 512], f32, name="gt")
                nc.vector.tensor_scalar_mul(out=gt[:], in0=psb[:],
                                            scalar1=rc16[:, 0:1])
                fc1c = []
                for c in range(4):
                    fw = fsb.tile([P, 32], f32, name=f"fc1c{c}")
                    nc.sync.dma_start(out=fw[:], in_=t_fc1w[c * P:(c + 1) * P, :])
                    fc1c.append(fw)
                fb1 = fsb.tile([32, 1], f32, name="fb1")
                nc.sync.dma_start(out=fb1[:], in_=t_fc1b[:])
                fw2 = fsb.tile([32, 10], f32, name="fw2")
                nc.sync.dma_start(out=fw2[:], in_=t_fc2w[:])
                fb2 = fsb.tile([16, 10], f32, name="fb2")
                nc.sync.dma_start(out=fb2[:], in_=t_fc2br[:])

                fc1_ps = fps.tile([32, 16], f32, name="fc1_ps")
                for c in range(4):
                    gtt_ps = fps.tile([P, 16], f32, name="gtt_ps", tag="gtt")
                    nc.tensor.transpose(out=gtt_ps[:], in_=gt[:, c * P:(c + 1) * P],
                                        identity=ident[0:16, 0:16])
                    gtt = fsb.tile([P, 16], f32, name="gtt_sb", tag="gtts")
                    nc.vector.tensor_copy(out=gtt[:], in_=gtt_ps[:])
                    nc.tensor.matmul(fc1_ps[:], lhsT=fc1c[c][:],
                                     rhs=gtt[:],
                                     start=(c == 0), stop=(c == 3))
                y1 = fsb.tile([32, 16], f32, name="y1")
                nc.vector.tensor_scalar_add(out=y1[:], in0=fc1_ps[:],
                                            scalar1=fb1[:, 0:1])
                neg1 = fsb.tile([32, 16], f32, name="neg1")
                nc.vector.tensor_scalar_min(out=neg1[:], in0=y1[:], scalar1=0.0)
                en1 = fsb.tile([32, 16], f32, name="en1")
                nc.scalar.activation(en1[:], neg1[:], AF.Exp)
                pm11 = fsb.tile([32, 16], f32, name="pm11")
                nc.vector.tensor_scalar(out=pm11[:], in0=y1[:], scalar1=0.0,
                                        scalar2=-1.0, op0=OP.max, op1=OP.add)
                g2 = fsb.tile([32, 16], f32, name="g2")
                nc.vector.tensor_add(out=g2[:], in0=pm11[:], in1=en1[:])

                fc2_ps = fps.tile([16, 10], f32, name="fc2_ps")
                nc.tensor.matmul(fc2_ps[:], lhsT=g2[:], rhs=fw2[:],
                                 start=True, stop=True)
                osb = fsb.tile([16, 10], f32, name="osb")
                nc.vector.tensor_add(out=osb[:], in0=fc2_ps[:], in1=fb2[:])
                nc.sync.dma_start(out=t_out[:], in_=osb[:])

    nc.compile()
    return nc


def kernel(x, edge_index, batch, W1, att_src1, att_dst1, b1,
           W2, att_src2, att_dst2, b2, fc1_w, fc1_b, fc2_w, fc2_b,
           _trace=False):
    from concourse.bass_utils import run_bass_kernel_spmd
    if _trace:
        try:
            import profile_util
            profile_util.install()
        except Exception:
            pass

    x = np.asarray(x, np.float32)
    W1 = np.asarray(W1, np.float32)
    W2 = np.asarray(W2, np.float32)
    a_s1 = np.asarray(att_src1, np.float32)
    a_d1 = np.asarray(att_dst1, np.float32)
    a_s2 = np.asarray(att_src2, np.float32)
    a_d2 = np.asarray(att_dst2, np.float32)

    pp = _preprocess(np.asarray(edge_index), np.asarray(batch))
    bt, btA, btB = pp['bt'], pp['btA'], pp['btB']

    key = (bt, btA, btB)
    if key not in _PROGRAM_CACHE:
        _PROGRAM_CACHE[key] = _build_program(bt, btA, btB)
    nc = _PROGRAM_CACHE[key]

    x_pad = np.zeros((NPAD, P), np.float32)
    x_pad[:N] = x
    x16 = x_pad.astype(np.float16)
    xT16 = np.ascontiguousarray(x16.T)
    V1 = np.zeros((P, 16), np.float32)
    V2 = np.zeros((1024, 16), np.float32)
    for h in range(8):
        V1[:, h] = W1[:, h * P:(h + 1) * P] @ a_s1[h]
        V1[:, 8 + h] = W1[:, h * P:(h + 1) * P] @ a_d1[h]
        V2[:, h] = W2[:, h * 64:(h + 1) * 64] @ a_s2[h]
        V2[:, 8 + h] = W2[:, h * 64:(h + 1) * 64] @ a_d2[h]

    b1c = np.asarray(b1, np.float32).reshape(8, P).T.copy()

    common = {
        "x16": x16,
        "xT16": xT16,
        "W1_16": W1.astype(np.float16),
        "V1_16": V1.astype(np.float16),
        "W2_16": W2.astype(np.float16),
        "V2_16": V2.astype(np.float16),
        "b1cols": b1c,
        "b2rep16": np.tile(np.asarray(b2, np.float16)[None, :], (P, 1)),
        "iota128_16": np.tile(np.arange(P, dtype=np.float16)[None, :], (P, 1)),
        "iota16_16": np.tile(np.arange(16, dtype=np.float16)[None, :], (P, 1)),
        "iotacol": np.arange(P, dtype=np.float32).reshape(P, 1),
        "recip_cnt16": pp['recip_cnt16'],
        "fc1_w": np.asarray(fc1_w, np.float32),
        "fc1_b": np.asarray(fc1_b, np.float32).reshape(32, 1),
        "fc2_w": np.asarray(fc2_w, np.float32),
        "fc2_b_rep": np.tile(np.asarray(fc2_b, np.float32)[None, :], (16, 1)),
    }
    in_maps = []
    for c in range(NCORES):
        m = dict(common)
        m["src_m"] = pp['src_m'][c]
        m["seg_m"] = pp['seg_m'][c]
        m["segT_m"] = pp['segT_m'][c]
        m["srcA_m"] = pp['srcA_m'][c]
        m["segA_m"] = pp['segA_m'][c]
        m["segTA_m"] = pp['segTA_m'][c]
        m["srcB_m"] = pp['srcB_m'][c]
        m["segB_m"] = pp['segB_m'][c]
        m["segTB_m"] = pp['segTB_m'][c]
        m["dsttile"] = pp['dsttile'][c]
        m["gid_m"] = pp['gid'][c]
        m["chain"] = np.zeros((16, 10), np.float32)
        in_maps.append(m)

    res = run_bass_kernel_spmd(nc, in_maps, list(range(NCORES)),
                               trace=bool(_trace))
    LAST_PROFILE.clear()
    LAST_PROFILE['exec_time_ns'] = res.exec_time_ns
    LAST_PROFILE['results'] = res
    return np.asarray(res.results[0]["out"], np.float32)


# revision 12
# speedup vs baseline: 1.8854x; 1.0496x over previous
"""GAT (2x GATConv + global_mean_pool + MLP) on 8 Trainium2 NeuronCores.

Strategy (sharding_hint: 1D node partition, replicated weights):
  - dst nodes partitioned 8 ways (1250/core, 10 tiles of 128 slots);
    edges sorted by dst, packed into per-(core,tile) blocks of 128.
  - Layer 1 aggregates x[src] (128 wide) instead of h[src] (1024 wide):
    sum_e ex*(x W1) == (sum_e ex*x) W1 per head -> 8x less gather traffic.
    Attention logits via fused vectors V = W @ a (alpha = x @ V).
  - All aggregation math in fp16 (PE 4x faster than f32; gathers half the
    bytes); PSUM accumulation stays f32.
  - alpha_dst is NOT gathered per edge: dst rows of a tile are contiguous,
    so fetch the tile's [128,8] alphas once and permute per block with a
    one-hot matmul (ohT built from a host-side transposed segment table
    via a rank-1 broadcast matmul + is_equal, batched 4 blocks at a time).
  - Vector/scalar engines split the per-head msg multiplies and
    normalization (4 heads each) to balance load.
  - Only exchange: AllGather of per-core [1280, 528] fp16 packed table
    (h2 = elu(out1) @ W2 | alpha2_src | alpha2_dst), done in 2 halves so
    the first overlaps the second half of layer-1 compute; plus a tiny
    AllReduce of pooled per-graph sums. MLP replicated.
"""
import os
import sys
import numpy as np

for _p in ("/opt/trn_rl_repo",):
    if os.path.isdir(_p) and _p not in sys.path:
        sys.path.insert(0, _p)

N = 10000
B = 16
NCORES = 8
P = 128
NPC = 1250                  # nodes per core
TPC = 10                    # dst tiles per core
NPAD = 10112                # 79 * 128
NTILES_A = 79
NEG = 0.2
L2ROWS = NCORES * TPC * P   # 10240
XAW = 144                   # xa row: 128 x | 8 a_src | 8 a_dst (f16)
H2W = 528                   # h2a row: 512 h2 | 8 a2src | 8 a2dst (f16)
HROWS = TPC * P // 2        # 640 rows per AllGather half
G = 3                       # blocks per inner group

_PROGRAM_CACHE = {}
LAST_PROFILE = {}
DEBUG_DUMPS = False


def _preprocess(edge_index, batch):
    src = np.concatenate([np.asarray(edge_index[0]), np.arange(N)]).astype(np.int64)
    dst = np.concatenate([np.asarray(edge_index[1]), np.arange(N)]).astype(np.int64)
    order = np.argsort(dst, kind='stable')
    src, dst = src[order], dst[order]

    core_of = dst // NPC
    local = dst - core_of * NPC
    tile_of = local // P
    seg_of = (local - tile_of * P).astype(np.float32)

    counts = np.zeros((NCORES, TPC), dtype=np.int64)
    np.add.at(counts, (core_of, tile_of), 1)
    bt = int(np.ceil(counts.max() / P))
    bt = max(bt, 1)

    src_m = np.zeros((NCORES, TPC, P, bt), dtype=np.int32)
    seg_m = np.full((NCORES, TPC, P, bt), -1.0, dtype=np.float32)

    flat_group = core_of * TPC + tile_of
    grp_start = np.searchsorted(flat_group, np.arange(NCORES * TPC), 'left')
    rank = np.arange(len(flat_group)) - grp_start[flat_group]
    blk = rank // P
    part = rank % P
    co = core_of.astype(np.int64)
    ti = tile_of.astype(np.int64)
    src_m[co, ti, part, blk] = src.astype(np.int32)
    seg_m[co, ti, part, blk] = seg_of

    # transposed segment table: segT[c, t, b*128 + j] = seg_m[c, t, j, b]
    segT_m = np.ascontiguousarray(
        seg_m.transpose(0, 1, 3, 2).reshape(NCORES, TPC, bt * P)).astype(np.float16)

    # L2 rows, split into two half-tables for the 2-phase AllGather:
    # node n on core c, local r in [0,1280): half = r//640,
    # row within its half-table = c*640 + (r - half*640)
    node = np.arange(N, dtype=np.int64)
    cn = node // NPC
    r = node - cn * NPC
    half_of = (r // HROWS).astype(np.int64)
    l2half = (cn * HROWS + (r - half_of * HROWS)).astype(np.int32)

    # partition each (core,tile)'s edges by source half; repack blocks
    srcA = [[None] * TPC for _ in range(NCORES)]
    btA = btB = 1
    packs = []
    for c in range(NCORES):
        for t in range(TPC):
            m = (co == c) & (ti == t)
            s_, g_ = src[m], seg_of[m]
            hmask = half_of[s_] == 0
            packs.append((c, t, s_[hmask], g_[hmask], s_[~hmask], g_[~hmask]))
            btA = max(btA, int(np.ceil(hmask.sum() / P)))
            btB = max(btB, int(np.ceil((~hmask).sum() / P)))

    def _pack(npart, s_, g_):
        sm = np.zeros((P, npart), dtype=np.int32)
        gm = np.full((P, npart), -1.0, dtype=np.float32)
        k = np.arange(len(s_))
        sm[k % P, k // P] = l2half[s_]
        gm[k % P, k // P] = g_
        return sm, gm

    srcA_m = np.zeros((NCORES, TPC, P, btA), dtype=np.int32)
    segA_m = np.full((NCORES, TPC, P, btA), -1.0, dtype=np.float32)
    srcB_m = np.zeros((NCORES, TPC, P, btB), dtype=np.int32)
    segB_m = np.full((NCORES, TPC, P, btB), -1.0, dtype=np.float32)
    for c, t, sA, gA, sB, gB in packs:
        srcA_m[c, t], segA_m[c, t] = _pack(btA, sA, gA)
        srcB_m[c, t], segB_m[c, t] = _pack(btB, sB, gB)
    segTA_m = np.ascontiguousarray(
        segA_m.transpose(0, 1, 3, 2).reshape(NCORES, TPC, btA * P)).astype(np.float16)
    segTB_m = np.ascontiguousarray(
        segB_m.transpose(0, 1, 3, 2).reshape(NCORES, TPC, btB * P)).astype(np.float16)

    # per-tile dst node ids (rows of xa_tab): core c, tile t, slot p
    dsttile = np.zeros((NCORES, TPC, P, 1), dtype=np.int32)
    for c in range(NCORES):
        for t in range(TPC):
            ids = c * NPC + t * P + np.arange(P)
            ids = np.minimum(ids, (c + 1) * NPC - 1)   # clamp pad slots
            dsttile[c, t, :, 0] = ids

    batch = np.asarray(batch).astype(np.int64)
    gid = np.full((NCORES, TPC, P), -1.0, dtype=np.float32)
    for c in range(NCORES):
        lo = c * NPC
        hi = min(lo + NPC, N)
        vals = batch[lo:hi].astype(np.float32)
        g = gid[c].reshape(-1)
        g[:hi - lo] = vals

    cnt = np.zeros(B, np.float32)
    np.add.at(cnt, batch, 1.0)
    recip_cnt16 = (1.0 / np.maximum(cnt, 1.0)).astype(np.float32).reshape(16, 1)

    return dict(bt=bt, btA=btA, btB=btB, src_m=src_m, seg_m=seg_m,
                segT_m=segT_m, srcA_m=srcA_m, segA_m=segA_m, segTA_m=segTA_m,
                srcB_m=srcB_m, segB_m=segB_m, segTB_m=segTB_m,
                dsttile=dsttile, gid=gid, recip_cnt16=recip_cnt16)


def _build_program(bt, btA, btB, upto='full', repeat=1):
    import concourse.bacc as bacc
    import concourse.bass as bass
    import concourse.mybir as mybir
    import concourse.tile as tile
    from concourse.masks import make_identity

    f32 = mybir.dt.float32
    f16 = mybir.dt.float16
    i32 = mybir.dt.int32
    AF = mybir.ActivationFunctionType
    OP = mybir.AluOpType
    IOA = bass.IndirectOffsetOnAxis

    _ORDER = ['none', 'A', 'B', 'AG', 'E', 'AR', 'F', 'full']
    def _inc(s):
        return _ORDER.index(upto if upto != 'full' else 'F') >= _ORDER.index(s)

    nc = bacc.Bacc("TRN2", target_bir_lowering=False, debug=False,
                   enable_asserts=False, num_devices=NCORES)

    def mm_noldw(*args, **kw):
        # matmul that reuses the PE weights loaded by the immediately
        # preceding matmul (same lhsT, consecutive in the PE stream)
        i = nc.tensor.matmul(*args, **kw)
        i.ins.ldweights = False
        return i

    # ---------------- inputs ----------------
    t_x16 = nc.dram_tensor("x16", [NPAD, P], f16, kind="ExternalInput")
    t_xT16 = nc.dram_tensor("xT16", [P, NPAD], f16, kind="ExternalInput")
    t_W1 = nc.dram_tensor("W1_16", [P, 1024], f16, kind="ExternalInput")
    t_V1 = nc.dram_tensor("V1_16", [P, 16], f16, kind="ExternalInput")
    t_W2 = nc.dram_tensor("W2_16", [1024, 512], f16, kind="ExternalInput")
    t_V2 = nc.dram_tensor("V2_16", [1024, 16], f16, kind="ExternalInput")
    t_b1c = nc.dram_tensor("b1cols", [P, 8], f32, kind="ExternalInput")
    t_b2r = nc.dram_tensor("b2rep16", [P, 512], f16, kind="ExternalInput")
    t_iota128 = nc.dram_tensor("iota128_16", [P, P], f16, kind="ExternalInput")
    t_iota16 = nc.dram_tensor("iota16_16", [P, 16], f16, kind="ExternalInput")
    t_iotacol = nc.dram_tensor("iotacol", [P, 1], f32, kind="ExternalInput")
    t_rc16 = nc.dram_tensor("recip_cnt16", [16, 1], f32, kind="ExternalInput")
    t_fc1w = nc.dram_tensor("fc1_w", [512, 32], f32, kind="ExternalInput")
    t_fc1b = nc.dram_tensor("fc1_b", [32, 1], f32, kind="ExternalInput")
    t_fc2w = nc.dram_tensor("fc2_w", [32, 10], f32, kind="ExternalInput")
    t_fc2br = nc.dram_tensor("fc2_b_rep", [16, 10], f32, kind="ExternalInput")
    t_srcm = nc.dram_tensor("src_m", [TPC, P, bt], i32, kind="ExternalInput")
    t_segm = nc.dram_tensor("seg_m", [TPC, P, bt], f32, kind="ExternalInput")
    t_segT = nc.dram_tensor("segT_m", [TPC, bt * P], f16, kind="ExternalInput")
    t_srcA = nc.dram_tensor("srcA_m", [TPC, P, btA], i32, kind="ExternalInput")
    t_segA = nc.dram_tensor("segA_m", [TPC, P, btA], f32, kind="ExternalInput")
    t_segTA = nc.dram_tensor("segTA_m", [TPC, btA * P], f16, kind="ExternalInput")
    t_srcB = nc.dram_tensor("srcB_m", [TPC, P, btB], i32, kind="ExternalInput")
    t_segB = nc.dram_tensor("segB_m", [TPC, P, btB], f32, kind="ExternalInput")
    t_segTB = nc.dram_tensor("segTB_m", [TPC, btB * P], f16, kind="ExternalInput")
    t_dstt = nc.dram_tensor("dsttile", [TPC, P, 1], i32, kind="ExternalInput")
    t_gid = nc.dram_tensor("gid_m", [TPC, P], f32, kind="ExternalInput")
    t_chain = nc.dram_tensor("chain", [16, 10], f32, kind="ExternalInput")

    t_out = nc.dram_tensor("out", [16, 10], f32, kind="ExternalOutput")
    dbg = {}
    if DEBUG_DUMPS:
        dbg['xa'] = nc.dram_tensor("dbg_xa", [NPAD, XAW], f16, kind="ExternalOutput")
        dbg['h2a'] = nc.dram_tensor("dbg_h2a", [TPC * P, H2W], f16, kind="ExternalOutput")
        dbg['pool'] = nc.dram_tensor("dbg_pool", [16, 512], f32, kind="ExternalOutput")

    def groups(n):
        out, b = [], 0
        while b < n:
            out.append(list(range(b, min(b + G, n))))
            b += G
        return out

    with tile.TileContext(nc) as tc:
        with (
            tc.tile_pool(name="const", bufs=1) as csb,
            tc.tile_pool(name="dram", bufs=1, space="DRAM") as dr,
        ):
            # DRAM staging
            xa_tab = dr.tile([NPAD, XAW], f16)          # [x | a_src | a_dst]
            h2a_loc = dr.tile([TPC * P, H2W], f16)      # [h2 | a2src | a2dst]
            pool_in = dr.tile([16, 512], f32)

            # resident constants
            ident = csb.tile([P, P], f32)
            make_identity(nc, ident[:])
            ident16 = csb.tile([P, P], f16)
            make_identity(nc, ident16[:])
            ones1 = csb.tile([1, P], f16)
            nc.vector.memset(ones1[:], 1.0)
            iota128 = csb.tile([P, P], f16)
            nc.sync.dma_start(out=iota128[:], in_=t_iota128[:])
            iota16 = csb.tile([P, 16], f16)
            nc.sync.dma_start(out=iota16[:], in_=t_iota16[:])
            iotacol = csb.tile([P, 1], f32)
            nc.sync.dma_start(out=iotacol[:], in_=t_iotacol[:])
            W1sb = csb.tile([P, 1024], f16)
            nc.sync.dma_start(out=W1sb[:], in_=t_W1[:])
            V1sb = csb.tile([P, 16], f16)
            nc.sync.dma_start(out=V1sb[:], in_=t_V1[:])
            W2sb = []
            V2sb = []
            for c in range(8):
                w2c = csb.tile([P, 512], f16, name=f"w2c{c}")
                nc.sync.dma_start(out=w2c[:], in_=t_W2[c * P:(c + 1) * P, :])
                W2sb.append(w2c)
                v2c = csb.tile([P, 16], f16, name=f"v2c{c}")
                nc.sync.dma_start(out=v2c[:], in_=t_V2[c * P:(c + 1) * P, :])
                V2sb.append(v2c)
            b1c = csb.tile([P, 8], f32)
            nc.sync.dma_start(out=b1c[:], in_=t_b1c[:])
            b2r = csb.tile([P, 512], f16)
            nc.sync.dma_start(out=b2r[:], in_=t_b2r[:])
            rc16 = csb.tile([16, 1], f32)
            nc.sync.dma_start(out=rc16[:], in_=t_rc16[:])
            chain_sb = csb.tile([16, 10], f32)
            nc.sync.dma_start(out=chain_sb[:], in_=t_chain[:])

            for _rep in range(repeat):
                h2a_A = dr.tile([NCORES * HROWS, H2W], f16, addr_space="Shared",
                                name=f"h2a_A{_rep}")
                h2a_B = dr.tile([NCORES * HROWS, H2W], f16, addr_space="Shared",
                                name=f"h2a_B{_rep}")
                pool_out = dr.tile([16, 512], f32, addr_space="Shared",
                                   name=f"pool_out{_rep}")
                # ---------------- Phase A: xa table (replicated) --------------
                with (
                    tc.tile_pool(name="pa_xt", bufs=1) as axt,
                    tc.tile_pool(name="pa_sb", bufs=3) as asb,
                    tc.tile_pool(name="pa_ps", bufs=3, space="PSUM") as aps,
                ):
                    xTsb = axt.tile([P, NPAD], f16, name="xTsb")
                    nc.sync.dma_start(out=xTsb[:], in_=t_xT16[:])
                    CH = 8      # tiles per batched table write
                    t0 = 0
                    while _inc('A') and t0 < NTILES_A:
                        csz = min(CH, NTILES_A - t0)
                        mrg8 = asb.tile([P, CH * XAW], f16, name="mrg8")
                        for i in range(csz):
                            t = t0 + i
                            sl = slice(t * P, (t + 1) * P)
                            tp_ = aps.tile([P, P], f16, name="atp", tag="atp")
                            nc.tensor.transpose(out=tp_[:], in_=xTsb[:, sl],
                                                identity=ident16[:])
                            al_ps = aps.tile([P, 16], f32, name="al_ps", tag="al")
                            nc.tensor.matmul(al_ps[:], lhsT=xTsb[:, sl],
                                             rhs=V1sb[:], start=True, stop=True)
                            o = i * XAW
                            nc.vector.tensor_copy(out=mrg8[:, o:o + P], in_=tp_[:])
                            nc.vector.tensor_copy(out=mrg8[:, o + P:o + P + 16],
                                                  in_=al_ps[:])
                        dst_v = xa_tab[t0 * P:(t0 + csz) * P, :].rearrange(
                            "(u p) w -> p u w", u=csz)
                        src_v = mrg8[:, 0:csz * XAW].rearrange(
                            "p (u w) -> p u w", u=csz)
                        nc.sync.dma_start(out=dst_v, in_=src_v)
                        t0 += csz

                # ---------------- Phase B: L1 aggregation + finalize ----------
                with (
                    tc.tile_pool(name="pb_sb", bufs=4) as bsb,
                    tc.tile_pool(name="pb_big", bufs=3) as bbig,
                    tc.tile_pool(name="pb_fin", bufs=2) as bfin,
                    tc.tile_pool(name="pb_ps", bufs=1, space="PSUM") as bps,
                ):
                    for t in range(TPC if _inc('B') else 0):
                        srcm = bsb.tile([P, bt], i32, name="srcm")
                        nc.sync.dma_start(out=srcm[:], in_=t_srcm[t])
                        segm = bsb.tile([P, bt], f32, name="segm")
                        nc.sync.dma_start(out=segm[:], in_=t_segm[t])
                        segT = bsb.tile([1, bt * P], f16, name="segT")
                        nc.sync.dma_start(out=segT[:], in_=t_segT[t, None, :])
                        dstt = bsb.tile([P, 1], i32, name="dstt")
                        nc.sync.dma_start(out=dstt[:], in_=t_dstt[t])
                        aD1t = bsb.tile([P, XAW], f16, name="aD1t")
                        nc.gpsimd.indirect_dma_start(
                            out=aD1t[:], out_offset=None, in_=xa_tab[:],
                            in_offset=IOA(ap=dstt[:, 0:1], axis=0))

                        # axden cols: 0:1024 num, 1024:1032 den, 1040:1072 edst
                        axden = bps.tile([P, 1536], f32, name="axden", bufs=1)
                        for grp in groups(bt):
                            g0, gsz = grp[0], len(grp)
                            xag = bbig.tile([P, G * XAW], f16, name="xag")
                            for k, b in enumerate(grp):
                                nc.gpsimd.indirect_dma_start(
                                    out=xag[:, k * XAW:(k + 1) * XAW],
                                    out_offset=None, in_=xa_tab[:],
                                    in_offset=IOA(ap=srcm[:, b:b + 1], axis=0))
                            sged = bps.tile([P, G * P + G * 8], f32,
                                            name="sged", tag="sged", bufs=1)
                            nc.tensor.matmul(
                                sged[:, 0:gsz * P], lhsT=ones1[:],
                                rhs=segT[0:1, g0 * P:(g0 + gsz) * P],
                                start=True, stop=True)
                            ohT = bbig.tile([P, G * P], f16, name="ohT")
                            nc.vector.tensor_scalar(
                                out=ohT[:, 0:gsz * P], in0=sged[:, 0:gsz * P],
                                scalar1=iotacol[:, 0:1],
                                scalar2=None, op0=OP.is_equal)
                            for k in range(gsz):
                                nc.tensor.matmul(
                                    sged[:, G * P + k * 8:G * P + (k + 1) * 8],
                                    lhsT=ohT[:, k * P:(k + 1) * P],
                                    rhs=aD1t[:, XAW - 8:XAW],
                                    start=True, stop=True, skip_group_check=True)
                            e4 = bsb.tile([P, G * 8], f16, name="e4")
                            asrc_v = xag[:].rearrange("p (k w) -> p k w", k=G)
                            nc.vector.tensor_tensor(
                                out=e4[:, 0:gsz * 8].rearrange("p (k w) -> p k w", k=gsz),
                                in0=sged[:, G * P:G * P + gsz * 8].rearrange(
                                    "p (k w) -> p k w", k=gsz),
                                in1=asrc_v[:, 0:gsz, P:P + 8], op=OP.add)
                            es4 = bsb.tile([P, G * 8], f16, name="es4")
                            nc.vector.tensor_scalar_mul(out=es4[:, 0:gsz * 8],
                                                        in0=e4[:, 0:gsz * 8],
                                                        scalar1=NEG)
                            lr4 = bsb.tile([P, G * 8], f16, name="lr4")
                            nc.vector.tensor_max(out=lr4[:, 0:gsz * 8],
                                                 in0=e4[:, 0:gsz * 8],
                                                 in1=es4[:, 0:gsz * 8])
                            ex4 = bsb.tile([P, G * 8], f32, name="ex4")
                            nc.scalar.activation(ex4[:, 0:gsz * 8],
                                                 lr4[:, 0:gsz * 8], AF.Exp)
                            ex16 = bsb.tile([P, G * 8], f16, name="ex16")
                            nc.vector.tensor_copy(out=ex16[:, 0:gsz * 8],
                                                  in_=ex4[:, 0:gsz * 8])

                            for k, b in enumerate(grp):
                                oh = bbig.tile([P, P], f16, name="oh")
                                nc.vector.tensor_scalar(
                                    out=oh[:], in0=iota128[:],
                                    scalar1=segm[:, b:b + 1],
                                    scalar2=None, op0=OP.is_equal)
                                msg = bbig.tile([P, 1024], f16, name="msg")
                                xs = xag[:, k * XAW:k * XAW + P]
                                for h in range(7):
                                    nc.vector.tensor_scalar_mul(
                                        out=msg[:, h * P:(h + 1) * P], in0=xs,
                                        scalar1=ex4[:, k * 8 + h:k * 8 + h + 1])
                                for h in range(7, 8):
                                    nc.scalar.activation(
                                        msg[:, h * P:(h + 1) * P], xs, AF.Copy,
                                        scale=ex4[:, k * 8 + h:k * 8 + h + 1])
                                st = (b == 0)
                                sp = (b == bt - 1)
                                nc.tensor.matmul(axden[:, 0:512], lhsT=oh[:],
                                                 rhs=msg[:, 0:512],
                                                 start=st, stop=sp)
                                mm_noldw(axden[:, 512:1024], lhsT=oh[:],
                                         rhs=msg[:, 512:1024],
                                         start=st, stop=sp)
                                mm_noldw(axden[:, 1024:1032], lhsT=oh[:],
                                         rhs=ex16[:, k * 8:(k + 1) * 8],
                                         start=st, stop=sp)

                        # ---- finalize tile t
                        den = bsb.tile([P, 8], f32, name="den")
                        nc.vector.tensor_scalar_max(out=den[:],
                                                    in0=axden[:, 1024:1032],
                                                    scalar1=1e-30)
                        rec = bsb.tile([P, 8], f32, name="rec")
                        nc.vector.reciprocal(out=rec[:], in_=den[:])
                        # normalize (4 heads DVE, 4 heads scalar)
                        axn = bfin.tile([P, 1024], f16, name="axn")
                        for h in range(4):
                            hs = slice(h * P, (h + 1) * P)
                            nc.vector.tensor_scalar_mul(out=axn[:, hs],
                                                        in0=axden[:, hs],
                                                        scalar1=rec[:, h:h + 1])
                        for h in range(4, 8):
                            hs = slice(h * P, (h + 1) * P)
                            nc.scalar.activation(axn[:, hs], axden[:, hs],
                                                 AF.Copy, scale=rec[:, h:h + 1])
                        h2p_ps = bps.tile([P, 512], f32, name="h2p_ps", bufs=1)
                        a2_ps = bps.tile([P, 16], f32, name="a2_ps", bufs=1)
                        for h in range(8):
                            hs = slice(h * P, (h + 1) * P)
                            tps = bps.tile([P, P], f16, name="tps", tag="tp", bufs=1)
                            nc.tensor.transpose(out=tps[:], in_=axn[:, hs],
                                                identity=ident16[:])
                            tsb = bfin.tile([P, P], f16, name="tsb", tag="ts", bufs=3)
                            nc.vector.tensor_copy(out=tsb[:], in_=tps[:])
                            # yT_h = W1_h^T @ axnT_h  [f_out, s]
                            yT = bps.tile([P, P], f32, name="yT", tag="yt", bufs=1)
                            nc.tensor.matmul(yT[:], lhsT=W1sb[:, hs], rhs=tsb[:],
                                             start=True, stop=True)
                            pre = bfin.tile([P, P], f16, name="pre", tag="pr", bufs=2)
                            nc.vector.tensor_scalar_add(out=pre[:], in0=yT[:],
                                                        scalar1=b1c[:, h:h + 1])
                            # elu in transposed space
                            m0 = bfin.tile([P, P], f16, name="m0", tag="m0", bufs=2)
                            nc.vector.tensor_scalar_min(out=m0[:], in0=pre[:],
                                                        scalar1=0.0)
                            en = bfin.tile([P, P], f16, name="en", tag="en", bufs=2)
                            nc.scalar.activation(en[:], m0[:], AF.Exp)
                            pm1 = bfin.tile([P, P], f16, name="pm1", tag="pm", bufs=2)
                            nc.vector.tensor_scalar(out=pm1[:], in0=pre[:],
                                                    scalar1=0.0, scalar2=-1.0,
                                                    op0=OP.max, op1=OP.add)
                            e1T = bfin.tile([P, P], f16, name="e1T", tag="e1", bufs=3)
                            nc.vector.tensor_add(out=e1T[:], in0=pm1[:], in1=en[:])
                            nc.tensor.matmul(h2p_ps[:], lhsT=e1T[:], rhs=W2sb[h][:],
                                             start=(h == 0), stop=(h == 7))
                            mm_noldw(a2_ps[:], lhsT=e1T[:], rhs=V2sb[h][:],
                                     start=(h == 0), stop=(h == 7))
                        pk = bfin.tile([P, H2W], f16, name="pk")
                        nc.vector.tensor_copy(out=pk[:, 0:512], in_=h2p_ps[:])
                        nc.vector.tensor_copy(out=pk[:, 512:528], in_=a2_ps[:])
                        sl = slice(t * P, (t + 1) * P)
                        nc.sync.dma_start(out=h2a_loc[sl, :], in_=pk[:])

                        # first-half AllGather as soon as tiles 0-4 are done
                        if _inc('AG') and t == TPC // 2 - 1:
                            nc.gpsimd.collective_compute(
                                "AllGather", mybir.AluOpType.bypass,
                                replica_groups=[list(range(NCORES))],
                                ins=[h2a_loc[0:HROWS, :].opt()],
                                outs=[h2a_A[:].opt()])

                if _inc('AG'):
                    nc.gpsimd.collective_compute(
                        "AllGather", mybir.AluOpType.bypass,
                        replica_groups=[list(range(NCORES))],
                        ins=[h2a_loc[HROWS:2 * HROWS, :].opt()],
                        outs=[h2a_B[:].opt()])

                # ---------------- Phase E: L2 aggregation + pool --------------
                with (
                    tc.tile_pool(name="pe_sb", bufs=4) as esb,
                    tc.tile_pool(name="pe_big", bufs=3) as ebig,
                    tc.tile_pool(name="pe_fin", bufs=2) as efin,
                    tc.tile_pool(name="pe_ps", bufs=1, space="PSUM") as eps,
                ):
                    pool_ps = eps.tile([16, 512], f32, name="pool_ps", bufs=1)
                    nblk = btA + btB
                    for t in range(TPC if _inc('E') else 0):
                        passes = []
                        for pn, (t_src, t_seg, t_segT, btP, htab) in enumerate(
                                [(t_srcA, t_segA, t_segTA, btA, h2a_A),
                                 (t_srcB, t_segB, t_segTB, btB, h2a_B)]):
                            srcm = esb.tile([P, btP], i32, name=f"srcm2{pn}")
                            nc.sync.dma_start(out=srcm[:], in_=t_src[t])
                            segm = esb.tile([P, btP], f32, name=f"segm2{pn}")
                            nc.sync.dma_start(out=segm[:], in_=t_seg[t])
                            segT = esb.tile([1, btP * P], f16, name=f"segT2{pn}")
                            nc.sync.dma_start(out=segT[:], in_=t_segT[t, None, :])
                            passes.append((srcm, segm, segT, btP, htab))
                        aD2 = esb.tile([P, 8], f16, name="aD2")
                        sl = slice(t * P, (t + 1) * P)
                        nc.sync.dma_start(out=aD2[:], in_=h2a_loc[sl, 520:528])

                        # ahden cols: 0:512 num, 512:520 den, 528:560 edst
                        ahden = eps.tile([P, 1024], f32, name="ahden", bufs=2)
                        bglob = 0
                        for srcm, segm, segT, btP, htab in passes:
                            for grp in groups(btP):
                                g0, gsz = grp[0], len(grp)
                                hg = ebig.tile([P, G * H2W], f16, name="hg")
                                for k, b in enumerate(grp):
                                    nc.gpsimd.indirect_dma_start(
                                        out=hg[:, k * H2W:(k + 1) * H2W],
                                        out_offset=None, in_=htab[:],
                                        in_offset=IOA(ap=srcm[:, b:b + 1], axis=0))
                                sged = eps.tile([P, G * P + G * 8], f32,
                                                name="sged2", tag="sged2", bufs=1)
                                nc.tensor.matmul(
                                    sged[:, 0:gsz * P], lhsT=ones1[:],
                                    rhs=segT[0:1, g0 * P:(g0 + gsz) * P],
                                    start=True, stop=True)
                                ohT = ebig.tile([P, G * P], f16, name="ohT2")
                                nc.vector.tensor_scalar(
                                    out=ohT[:, 0:gsz * P], in0=sged[:, 0:gsz * P],
                                    scalar1=iotacol[:, 0:1],
                                    scalar2=None, op0=OP.is_equal)
                                for k in range(gsz):
                                    nc.tensor.matmul(
                                        sged[:, G * P + k * 8:G * P + (k + 1) * 8],
                                        lhsT=ohT[:, k * P:(k + 1) * P],
                                        rhs=aD2[:], start=True, stop=True,
                                        skip_group_check=True)
                                e4 = esb.tile([P, G * 8], f16, name="e42")
                                asrc_v = hg[:].rearrange("p (k w) -> p k w", k=G)
                                nc.vector.tensor_tensor(
                                    out=e4[:, 0:gsz * 8].rearrange(
                                        "p (k w) -> p k w", k=gsz),
                                    in0=sged[:, G * P:G * P + gsz * 8].rearrange(
                                        "p (k w) -> p k w", k=gsz),
                                    in1=asrc_v[:, 0:gsz, 512:520], op=OP.add)
                                es4 = esb.tile([P, G * 8], f16, name="es42")
                                nc.vector.tensor_scalar_mul(out=es4[:, 0:gsz * 8],
                                                            in0=e4[:, 0:gsz * 8],
                                                            scalar1=NEG)
                                lr4 = esb.tile([P, G * 8], f16, name="lr42")
                                nc.vector.tensor_max(out=lr4[:, 0:gsz * 8],
                                                     in0=e4[:, 0:gsz * 8],
                                                     in1=es4[:, 0:gsz * 8])
                                ex4 = esb.tile([P, G * 8], f32, name="ex42")
                                nc.scalar.activation(ex4[:, 0:gsz * 8],
                                                     lr4[:, 0:gsz * 8], AF.Exp)
                                ex16 = esb.tile([P, G * 8], f16, name="ex162")
                                nc.vector.tensor_copy(out=ex16[:, 0:gsz * 8],
                                                      in_=ex4[:, 0:gsz * 8])

                                for k, b in enumerate(grp):
                                    oh = ebig.tile([P, P], f16, name="oh2")
                                    nc.vector.tensor_scalar(
                                        out=oh[:], in0=iota128[:],
                                        scalar1=segm[:, b:b + 1],
                                        scalar2=None, op0=OP.is_equal)
                                    msg = ebig.tile([P, 512], f16, name="msg2")
                                    hs_ = hg[:, k * H2W:k * H2W + 512]
                                    for h in range(8):
                                        nc.vector.tensor_scalar_mul(
                                            out=msg[:, h * 64:(h + 1) * 64],
                                            in0=hs_[:, h * 64:(h + 1) * 64],
                                            scalar1=ex4[:, k * 8 + h:k * 8 + h + 1])
                                    st = (bglob == 0)
                                    sp = (bglob == nblk - 1)
                                    nc.tensor.matmul(ahden[:, 0:512], lhsT=oh[:],
                                                     rhs=msg[:], start=st, stop=sp)
                                    mm_noldw(ahden[:, 512:520], lhsT=oh[:],
                                             rhs=ex16[:, k * 8:(k + 1) * 8],
                                             start=st, stop=sp)
                                    bglob += 1

                        # ---- finalize tile t
                        den = esb.tile([P, 8], f32, name="den2")
                        nc.vector.tensor_scalar_max(out=den[:],
                                                    in0=ahden[:, 512:520],
                                                    scalar1=1e-30)
                        rec = esb.tile([P, 8], f32, name="rec2")
                        nc.vector.reciprocal(out=rec[:], in_=den[:])
                        y = efin.tile([P, 512], f16, name="yl2")
                        for h in range(8):
                            hs = slice(h * 64, (h + 1) * 64)
                            nc.vector.tensor_scalar_mul(out=y[:, hs],
                                                        in0=ahden[:, hs],
                                                        scalar1=rec[:, h:h + 1])
                        y2 = efin.tile([P, 512], f16, name="y2l2")
                        nc.vector.tensor_add(out=y2[:], in0=y[:], in1=b2r[:])
                        neg = efin.tile([P, 512], f16, name="negl2")
                        nc.vector.tensor_scalar_min(out=neg[:], in0=y2[:], scalar1=0.0)
                        en = efin.tile([P, 512], f16, name="enl2")
                        nc.scalar.activation(en[:], neg[:], AF.Exp)
                        pm1 = efin.tile([P, 512], f16, name="pm1l2")
                        nc.vector.tensor_scalar(out=pm1[:], in0=y2[:], scalar1=0.0,
                                                scalar2=-1.0, op0=OP.max, op1=OP.add)
                        e2t = efin.tile([P, 512], f16, name="e2t")
                        nc.vector.tensor_add(out=e2t[:], in0=pm1[:], in1=en[:])

                        gidt = esb.tile([P, 1], f32, name="gidt")
                        nc.sync.dma_start(out=gidt[:], in_=t_gid[t, :, None])
                        gone = esb.tile([P, 16], f16, name="gone")
                        nc.vector.tensor_scalar(out=gone[:], in0=iota16[:],
                                                scalar1=gidt[:, 0:1], scalar2=None,
                                                op0=OP.is_equal)
                        nc.tensor.matmul(pool_ps[:], lhsT=gone[:], rhs=e2t[:],
                                         start=(t == 0), stop=(t == TPC - 1))

                    # pool -> dram bounce
                    if _inc('E'):
                        pool_sb = esb.tile([16, 512], f32, name="pool_sb")
                        nc.vector.tensor_copy(out=pool_sb[:], in_=pool_ps[:])
                        nc.sync.dma_start(out=pool_in[:], in_=pool_sb[:])

                if _inc('AR'):
                    nc.gpsimd.collective_compute(
                        "AllReduce", mybir.AluOpType.add,
                        replica_groups=[list(range(NCORES))],
                        ins=[pool_in[:].opt()], outs=[pool_out[:].opt()])

                if DEBUG_DUMPS:
                    nc.sync.dma_start(out=dbg['xa'][:], in_=xa_tab[:])
                    nc.sync.dma_start(out=dbg['h2a'][:], in_=h2a_loc[:])
                    nc.sync.dma_start(out=dbg['pool'][:], in_=pool_in[:])

            # ---------------- Phase F: MLP (replicated) -------------------
            if not _inc('F'):
                nc.sync.dma_start(out=t_out[:], in_=chain_sb[:])
            if _inc('F'):
             with (
                tc.tile_pool(name="pf_sb", bufs=1) as fsb,
                tc.tile_pool(name="pf_ps", bufs=1, space="PSUM") as fps,
             ):
                psb = fsb.tile([16, 512], f32, name="psb")
                nc.sync.dma_start(out=psb[:], in_=pool_out[:])
                gt = fsb.tile([16, 512], f32, name="gt")
                nc.vector.tensor_scalar_mul(out=gt[:], in0=psb[:],
                                            scalar1=rc16[:, 0:1])
                fc1c = []
                for c in range(4):
                    fw = fsb.tile([P, 32], f32, name=f"fc1c{c}")
                    nc.sync.dma_start(out=fw[:], in_=t_fc1w[c * P:(c + 1) * P, :])
                    fc1c.append(fw)
                fb1 = fsb.tile([32, 1], f32, name="fb1")
                nc.sync.dma_start(out=fb1[:], in_=t_fc1b[:])
                fw2 = fsb.tile([32, 10], f32, name="fw2")
                nc.sync.dma_start(out=fw2[:], in_=t_fc2w[:])
                fb2 = fsb.tile([16, 10], f32, name="fb2")
                nc.sync.dma_start(out=fb2[:], in_=t_fc2br[:])

                fc1_ps = fps.tile([32, 16], f32, name="fc1_ps")
                for c in range(4):
                    gtt_ps = fps.tile([P, 16], f32, name="gtt_ps", tag="gtt")
                    nc.tensor.transpose(out=gtt_ps[:], in_=gt[:, c * P:(c + 1) * P],
                                        identity=ident[0:16, 0:16])
                    gtt = fsb.tile([P, 16], f32, name="gtt_sb", tag="gtts")
                    nc.vector.tensor_copy(out=gtt[:], in_=gtt_ps[:])
                    nc.tensor.matmul(fc1_ps[:], lhsT=fc1c[c][:],
                                     rhs=gtt[:],
                                     start=(c == 0), stop=(c == 3))
                y1 = fsb.tile([32, 16], f32, name="y1")
                nc.vector.tensor_scalar_add(out=y1[:], in0=fc1_ps[:],
                                            scalar1=fb1[:, 0:1])
                neg1 = fsb.tile([32, 16], f32, name="neg1")
                nc.vector.tensor_scalar_min(out=neg1[:], in0=y1[:], scalar1=0.0)
                en1 = fsb.tile([32, 16], f32, name="en1")
                nc.scalar.activation(en1[:], neg1[:], AF.Exp)
                pm11 = fsb.tile([32, 16], f32, name="pm11")
                nc.vector.tensor_scalar(out=pm11[:], in0=y1[:], scalar1=0.0,
                                        scalar2=-1.0, op0=OP.max, op1=OP.add)
                g2 = fsb.tile([32, 16], f32, name="g2")
                nc.vector.tensor_add(out=g2[:], in0=pm11[:], in1=en1[:])

                fc2_ps = fps.tile([16, 10], f32, name="fc2_ps")
                nc.tensor.matmul(fc2_ps[:], lhsT=g2[:], rhs=fw2[:],
                                 start=True, stop=True)
                osb = fsb.tile([16, 10], f32, name="osb")
                nc.vector.tensor_add(out=osb[:], in0=fc2_ps[:], in1=fb2[:])
                nc.sync.dma_start(out=t_out[:], in_=osb[:])

    nc.compile()
    return nc


def kernel(x, edge_index, batch, W1, att_src1, att_dst1, b1,
           W2, att_src2, att_dst2, b2, fc1_w, fc1_b, fc2_w, fc2_b,
           _trace=False):
    from concourse.bass_utils import run_bass_kernel_spmd
    if _trace:
        try:
            import profile_util
            profile_util.install()
        except Exception:
            pass

    x = np.asarray(x, np.float32)
    W1 = np.asarray(W1, np.float32)
    W2 = np.asarray(W2, np.float32)
    a_s1 = np.asarray(att_src1, np.float32)
    a_d1 = np.asarray(att_dst1, np.float32)
    a_s2 = np.asarray(att_src2, np.float32)
    a_d2 = np.asarray(att_dst2, np.float32)

    pp = _preprocess(np.asarray(edge_index), np.asarray(batch))
    bt, btA, btB = pp['bt'], pp['btA'], pp['btB']

    key = (bt, btA, btB)
    if key not in _PROGRAM_CACHE:
        _PROGRAM_CACHE[key] = _build_program(bt, btA, btB)
    nc = _PROGRAM_CACHE[key]

    x_pad = np.zeros((NPAD, P), np.float32)
    x_pad[:N] = x
    x16 = x_pad.astype(np.float16)
    xT16 = np.ascontiguousarray(x16.T)
    V1 = np.zeros((P, 16), np.float32)
    V2 = np.zeros((1024, 16), np.float32)
    for h in range(8):
        V1[:, h] = W1[:, h * P:(h + 1) * P] @ a_s1[h]
        V1[:, 8 + h] = W1[:, h * P:(h + 1) * P] @ a_d1[h]
        V2[:, h] = W2[:, h * 64:(h + 1) * 64] @ a_s2[h]
        V2[:, 8 + h] = W2[:, h * 64:(h + 1) * 64] @ a_d2[h]

    b1c = np.asarray(b1, np.float32).reshape(8, P).T.copy()

    common = {
        "x16": x16,
        "xT16": xT16,
        "W1_16": W1.astype(np.float16),
        "V1_16": V1.astype(np.float16),
        "W2_16": W2.astype(np.float16),
        "V2_16": V2.astype(np.float16),
        "b1cols": b1c,
        "b2rep16": np.tile(np.asarray(b2, np.float16)[None, :], (P, 1)),
        "iota128_16": np.tile(np.arange(P, dtype=np.float16)[None, :], (P, 1)),
        "iota16_16": np.tile(np.arange(16, dtype=np.float16)[None, :], (P, 1)),
        "iotacol": np.arange(P, dtype=np.float32).reshape(P, 1),
        "recip_cnt16": pp['recip_cnt16'],
        "fc1_w": np.asarray(fc1_w, np.float32),
        "fc1_b": np.asarray(fc1_b, np.float32).reshape(32, 1),
        "fc2_w": np.asarray(fc2_w, np.float32),
        "fc2_b_rep": np.tile(np.asarray(fc2_b, np.float32)[None, :], (16, 1)),
    }
    in_maps = []
    for c in range(NCORES):
        m = dict(common)
        m["src_m"] = pp['src_m'][c]
        m["seg_m"] = pp['seg_m'][c]
        m["segT_m"] = pp['segT_m'][c]
        m["srcA_m"] = pp['srcA_m'][c]
        m["segA_m"] = pp['segA_m'][c]
        m["segTA_m"] = pp['segTA_m'][c]
        m["srcB_m"] = pp['srcB_m'][c]
        m["segB_m"] = pp['segB_m'][c]
        m["segTB_m"] = pp['segTB_m'][c]
        m["dsttile"] = pp['dsttile'][c]
        m["gid_m"] = pp['gid'][c]
        m["chain"] = np.zeros((16, 10), np.float32)
        in_maps.append(m)

    res = run_bass_kernel_spmd(nc, in_maps, list(range(NCORES)),
                               trace=bool(_trace))
    LAST_PROFILE.clear()
    LAST_PROFILE['exec_time_ns'] = res.exec_time_ns
    LAST_PROFILE['results'] = res
    return np.asarray(res.results[0]["out"], np.float32)
